# revision 19
# baseline (speedup 1.0000x reference)
"""Trainium2 Bass kernel for nn_ChainOfExperts (MoE with shared experts).

Strategy (8 NeuronCores):
  Phase pr (data-parallel, tokens sharded along B): router logits (f32 for
    exact top-2 agreement with the reference) + top-2 softmax weights.
  Host: pure data movement/layout — bf16 cast of x, bf16 cast of weights,
    group token slots by routed expert (counting sort on device-computed
    indices), gather bf16 token columns per expert.
  Phase p2w (expert-parallel, 2 experts per core): weight-stationary
    routed-expert FFN on the gathered tokens (capacity-padded); both
    experts' w1/w2 live in SBUF so the PE never stalls on weight DMA;
    output pre-scaled by combine weight.
  Phase p3s (data-parallel): shared-expert FFN (single M-batched layer-1
    composable over both shared experts, h kept in SBUF) fused with the
    final combine out = shared + y_slot0 + y_slot1.

All activations are kept feature-major ([D, tokens]) so every matmul has its
contraction dim on partitions. Matmuls run in bf16 (fp32 accumulate); the
router runs in fp32.
"""

import os
from contextlib import ExitStack
from dataclasses import dataclass

import numpy as np
import ml_dtypes

import concourse.bass as bass
import concourse.mybir as mybir
import concourse.tile as tile
from concourse import bacc
from concourse.bass import ts, ds
from concourse.bass_utils import run_bass_kernel_spmd
from concourse.kernels.tile_matmul import (
    ShapeInfo,
    composable_matmul_tile_kernel,
    cast_to_type,
    dma_from_dram_kxm,
    dma_from_dram_kxn,
    dma_to_dram_mxn,
)

BF16 = ml_dtypes.bfloat16
ActFn = mybir.ActivationFunctionType
dt = mybir.dt
P = 128

# bass_utils imports antenv.axon_hooks when tracing is requested; this
# container ships only an antenv stub. Provide the missing module so a
# trace request degrades to an untraced run instead of crashing.
import sys as _sys
try:
    import antenv.axon_hooks  # noqa: F401
except ImportError:
    import types as _types
    import antenv as _antenv
    _stub = _types.ModuleType("antenv.axon_hooks")
    _stub.get_axon_ntff_profile_hook = lambda: None
    _sys.modules["antenv.axon_hooks"] = _stub
    _antenv.axon_hooks = _stub


@dataclass(frozen=True)
class Cfg:
    n_cores: int = 8
    D: int = 2048     # hidden dim
    TPC: int = 2048   # tokens per core
    E: int = 16       # routed experts
    NSH: int = 2      # shared experts
    DS: int = 1024    # shared inner dim
    DR: int = 512     # routed inner dim
    CAP: int = 2304   # per-expert token capacity (multiple of 128)
    EPC: int = 2      # experts per core
    CAP_A: int = 2304  # slot-A capacity (8 most-loaded experts)
    CAP_B: int = 2048  # slot-B capacity (8 least-loaded experts)

    @property
    def n_tile(self):  # composable's N tiling for N=CAP
        return min(512, -(-self.CAP // P) * P)

    @property
    def n_tiles(self):
        return -(-self.CAP // self.n_tile)

    @property
    def CAPP(self):  # hg cache width: CAP padded to whole n-tiles
        return self.n_tiles * self.n_tile

    @property
    def W2(self):  # p2f token-slot width
        return self.CAP_A + self.CAP_B


CFG = Cfg()
TRACE = bool(os.environ.get("KERNEL_TRACE"))
LAST_EXEC_NS: dict[str, int | None] = {}

_cache: dict = {}


def _rearr2(ap):
    """[K, N] dram AP -> [pi, po, N] with K = po*128 + pi."""
    return ap.rearrange("(po pi) t -> pi po t", pi=P)


# --------------------------------------------------------------------------
# Phase 1: router + shared experts + bf16 cast of x
# --------------------------------------------------------------------------

class _NullCtx:
    def __enter__(self):
        return None

    def __exit__(self, *a):
        return False


def _maybe_loop(tc, loop_n):
    """Wrap the phase body in an in-NEFF repeat loop (for benchmarking)."""
    return tc.For_i(0, loop_n, 1) if loop_n else _NullCtx()


def build_p1(cfg: Cfg, debug: bool = False, loop_n: int = 0):
    nc = bacc.Bacc("TRN2", target_bir_lowering=False, debug=debug)
    f32 = dt.float32
    xT = nc.dram_tensor("xT", [cfg.D, cfg.TPC], f32, kind="ExternalInput").ap()
    rw = nc.dram_tensor("rw", [cfg.D, cfg.E], f32, kind="ExternalInput").ap()
    sw1 = nc.dram_tensor("sw1", [cfg.NSH, cfg.D, cfg.DS], f32, kind="ExternalInput").ap()
    sb1 = nc.dram_tensor("sb1", [cfg.NSH, cfg.DS], f32, kind="ExternalInput").ap()
    sw2 = nc.dram_tensor("sw2", [cfg.NSH, cfg.DS, cfg.D], f32, kind="ExternalInput").ap()
    sb2 = nc.dram_tensor("sb2", [cfg.NSH, cfg.D], f32, kind="ExternalInput").ap()
    out_shT = nc.dram_tensor("out_shT", [cfg.D, cfg.TPC], f32, kind="ExternalOutput").ap()
    xbfT = nc.dram_tensor("xbfT", [cfg.D, cfg.TPC], dt.bfloat16, kind="ExternalOutput").ap()
    ridx = nc.dram_tensor("ridx", [cfg.TPC, 8], dt.uint32, kind="ExternalOutput").ap()
    rwts = nc.dram_tensor("rwts", [cfg.TPC, 8], f32, kind="ExternalOutput").ap()
    h_dram = nc.dram_tensor("h_mid", [cfg.NSH, cfg.DS, cfg.TPC], dt.bfloat16).ap()

    x_po = cfg.D // P
    ds_po = cfg.DS // P
    CH = 256  # router/cast chunk (tokens)

    with tile.TileContext(nc) as tc, _maybe_loop(tc, loop_n), ExitStack() as ctx:
        const = ctx.enter_context(tc.tile_pool(name="const", bufs=1))
        rw_sb = const.tile([P, x_po, cfg.E], f32)
        nc.sync.dma_start(rw_sb[:], rw.rearrange("(po pi) e -> pi po e", pi=P))
        b1_sb = const.tile([P, cfg.NSH, ds_po], f32)
        nc.sync.dma_start(b1_sb[:], sb1.rearrange("s (po pi) -> pi s po", pi=P))
        b2_sb = const.tile([P, cfg.NSH, x_po], f32)
        nc.sync.dma_start(b2_sb[:], sb2.rearrange("s (po pi) -> pi s po", pi=P))
        b2sum = const.tile([P, x_po], f32)
        nc.vector.tensor_add(b2sum[:], b2_sb[:, 0], b2_sb[:, 1])
        xbf_cache = const.tile([P, x_po, cfg.TPC], dt.bfloat16)

        # ---- router + cast pass ----
        with ExitStack() as c2:
            xsrc = c2.enter_context(tc.tile_pool(name="xsrc", bufs=2))
            rps = c2.enter_context(tc.tile_pool(name="rpsum", bufs=2, space="PSUM"))
            rsb = c2.enter_context(tc.tile_pool(name="rsb", bufs=3))
            xT_t = _rearr2(xT)
            xbfT_t = _rearr2(xbfT)
            for c in range(cfg.TPC // CH):
                xt = xsrc.tile([P, x_po, CH], f32, tag="xt")
                nc.sync.dma_start(xt[:], xT_t[:, :, ts(c, CH)])
                nc.vector.tensor_copy(xbf_cache[:, :, ts(c, CH)], xt[:])
                nc.sync.dma_start(xbfT_t[:, :, ts(c, CH)], xbf_cache[:, :, ts(c, CH)])
                for tt in range(CH // P):
                    t0 = c * CH + tt * P
                    ps = rps.tile([P, cfg.E], f32, tag="rp")
                    for po in range(x_po):
                        nc.tensor.matmul(
                            ps[:], xt[:, po, ts(tt, P)], rw_sb[:, po, :],
                            start=(po == 0), stop=(po == x_po - 1),
                        )
                    lg = rsb.tile([P, cfg.E], f32, tag="lg")
                    nc.vector.tensor_copy(lg[:], ps[:])
                    mx = rsb.tile([P, 8], f32, tag="mx")
                    nc.vector.max(mx[:], lg[:])
                    ix = rsb.tile([P, 8], dt.uint32, tag="ix")
                    nc.vector.max_index(ix[:], mx[:], lg[:])
                    nm = rsb.tile([P, 1], f32, tag="nm")
                    nc.vector.tensor_scalar_mul(nm[:], mx[:, 0:1], -1.0)
                    ex = rsb.tile([P, cfg.E], f32, tag="ex")
                    zz = rsb.tile([P, 1], f32, tag="zz")
                    nc.scalar.activation(ex[:], lg[:], ActFn.Exp, bias=nm[:], accum_out=zz[:])
                    rz = rsb.tile([P, 1], f32, tag="rz")
                    nc.vector.reciprocal(rz[:], zz[:])
                    wv = rsb.tile([P, 8], f32, tag="wv")
                    nc.scalar.activation(wv[:], mx[:], ActFn.Exp, bias=nm[:])
                    nc.vector.tensor_scalar_mul(wv[:], wv[:], rz[:])
                    nc.sync.dma_start(ridx[ds(t0, P), :], ix[:])
                    nc.sync.dma_start(rwts[ds(t0, P), :], wv[:])

        # ---- shared experts layer 1 (per shared expert s) ----
        xbf_shape = ShapeInfo(pdims=((P, x_po),), fdims=(cfg.TPC,))

        def xbf_producer(nc_, md):
            return xbf_cache[
                :, ts(md.k_tile_idx, md.k_subtiles),
                ds(md.n_tile_idx * md.n_tile, md.n_tile)
            ]

        for s in range(cfg.NSH):
            with ExitStack() as c2:
                mpool = c2.enter_context(tc.tile_pool(name=f"l1m{s}", bufs=2))
                cpool = c2.enter_context(tc.tile_pool(name=f"l1c{s}", bufs=5))
                kxm_prod, kxm_shape = dma_from_dram_kxm(mpool, sw1[s])
                kxm_prod = cast_to_type(kxm_prod, cpool, dt.bfloat16)

                def l1_reducer(nc_, psum, sbuf, md, s=s):
                    ko = (md.m_tile_idx * md.m_tile + md.m_subtile_idx * P) // P
                    nc_.scalar.activation(
                        sbuf[:], psum[:], ActFn.Silu, bias=b1_sb[:, s, ko:ko + 1]
                    )

                composable_matmul_tile_kernel(
                    tc=tc,
                    kxm_shape=kxm_shape,
                    kxn_shape=xbf_shape,
                    output_type=dt.bfloat16,
                    kxm_producer=kxm_prod,
                    kxn_producer=xbf_producer,
                    mxn_consumer=dma_to_dram_mxn(h_dram[s]),
                    mxn_subtile_reducer=l1_reducer,
                )

        # ---- shared experts layer 2 (contract over s and DS jointly) ----
        with ExitStack() as c2:
            mpool = c2.enter_context(tc.tile_pool(name="l2m", bufs=2))
            cpool = c2.enter_context(tc.tile_pool(name="l2c", bufs=5))
            npool = c2.enter_context(tc.tile_pool(name="l2n", bufs=5))
            kxm_prod, kxm_shape = dma_from_dram_kxm(mpool, sw2, batch_k=True)
            kxm_prod = cast_to_type(kxm_prod, cpool, dt.bfloat16)
            kxn_prod, kxn_shape = dma_from_dram_kxn(npool, h_dram, batch_k=True)

            def l2_reducer(nc_, psum, sbuf, md):
                do = (md.m_tile_idx * md.m_tile + md.m_subtile_idx * P) // P
                nc_.vector.tensor_scalar_add(sbuf[:], psum[:], b2sum[:, do:do + 1])

            composable_matmul_tile_kernel(
                tc=tc,
                kxm_shape=kxm_shape,
                kxn_shape=kxn_shape,
                output_type=dt.float32,
                kxm_producer=kxm_prod,
                kxn_producer=kxn_prod,
                mxn_consumer=dma_to_dram_mxn(out_shT),
                mxn_subtile_reducer=l2_reducer,
            )

    nc.compile()
    return nc


def build_p1r(cfg: Cfg, debug: bool = False, loop_n: int = 0):
    """Router-only phase: top-2 indices/weights + bf16 cast of x.

    x is loaded po-sliced with full token rows (8KB contiguous runs) for DMA
    efficiency. Each 128-token group's logits use a private PSUM tile per
    po-slice (complete start/stop groups) and accumulate in SBUF, avoiding
    interleaved-group and PSUM bank-sharing hazards.
    """
    nc = bacc.Bacc("TRN2", target_bir_lowering=False, debug=debug)
    f32 = dt.float32
    xT = nc.dram_tensor("xT", [cfg.D, cfg.TPC], f32, kind="ExternalInput").ap()
    rw = nc.dram_tensor("rw", [cfg.D, cfg.E], f32, kind="ExternalInput").ap()
    xbfT = nc.dram_tensor("xbfT", [cfg.D, cfg.TPC], dt.bfloat16, kind="ExternalOutput").ap()
    ridx = nc.dram_tensor("ridx", [cfg.TPC, 8], dt.uint32, kind="ExternalOutput").ap()
    rwts = nc.dram_tensor("rwts", [cfg.TPC, 8], f32, kind="ExternalOutput").ap()

    x_po = cfg.D // P
    PO_CH = 2
    NPO = x_po // PO_CH
    NG = cfg.TPC // P

    with tile.TileContext(nc) as tc, _maybe_loop(tc, loop_n), ExitStack() as ctx:
        const = ctx.enter_context(tc.tile_pool(name="const", bufs=1))
        rw_sb = const.tile([P, x_po, cfg.E], f32)
        nc.sync.dma_start(rw_sb[:], rw.rearrange("(po pi) e -> pi po e", pi=P))
        lg_acc = const.tile([P, NG, cfg.E], f32)
        xsrc = ctx.enter_context(tc.tile_pool(name="xsrc", bufs=3))
        xbfp = ctx.enter_context(tc.tile_pool(name="xbfp", bufs=3))
        rps = ctx.enter_context(tc.tile_pool(name="rpsum", bufs=4, space="PSUM"))
        rsb = ctx.enter_context(tc.tile_pool(name="rsb", bufs=3))
        xT_t = _rearr2(xT)
        xbfT_t = _rearr2(xbfT)

        for po8 in range(NPO):
            xt = xsrc.tile([P, PO_CH, cfg.TPC], f32, tag="xt")
            nc.sync.dma_start(xt[:], xT_t[:, ts(po8, PO_CH), :])
            xb = xbfp.tile([P, PO_CH, cfg.TPC], dt.bfloat16, tag="xb")
            nc.vector.tensor_copy(xb[:], xt[:])
            nc.sync.dma_start(xbfT_t[:, ts(po8, PO_CH), :], xb[:])
            for tt in range(NG):
                ps = rps.tile([P, cfg.E], f32, tag="rp")
                for pp in range(PO_CH):
                    nc.tensor.matmul(
                        ps[:], xt[:, pp, ts(tt, P)],
                        rw_sb[:, po8 * PO_CH + pp, :],
                        start=(pp == 0), stop=(pp == PO_CH - 1),
                    )
                if po8 == 0:
                    nc.vector.tensor_copy(lg_acc[:, tt, :], ps[:])
                else:
                    nc.vector.tensor_add(lg_acc[:, tt, :], lg_acc[:, tt, :], ps[:])

        for tt in range(NG):
            t0 = tt * P
            lg = lg_acc[:, tt, :]
            mx = rsb.tile([P, 8], f32, tag="mx")
            nc.vector.max(mx[:], lg)
            ix = rsb.tile([P, 8], dt.uint32, tag="ix")
            nc.vector.max_index(ix[:], mx[:], lg)
            nm = rsb.tile([P, 1], f32, tag="nm")
            nc.vector.tensor_scalar_mul(nm[:], mx[:, 0:1], -1.0)
            ex = rsb.tile([P, cfg.E], f32, tag="ex")
            zz = rsb.tile([P, 1], f32, tag="zz")
            nc.scalar.activation(ex[:], lg, ActFn.Exp, bias=nm[:], accum_out=zz[:])
            rz = rsb.tile([P, 1], f32, tag="rz")
            nc.vector.reciprocal(rz[:], zz[:])
            wv = rsb.tile([P, 8], f32, tag="wv")
            nc.scalar.activation(wv[:], mx[:], ActFn.Exp, bias=nm[:])
            nc.vector.tensor_scalar_mul(wv[:], wv[:], rz[:])
            nc.sync.dma_start(ridx[ds(t0, P), :], ix[:])
            nc.sync.dma_start(rwts[ds(t0, P), :], wv[:])
    nc.compile()
    return nc


def _w_producer_batched(pool, w_ap, tagname):
    """Batched-K variant of _w_producer for [S, K, M] weights.

    f32 weights stream through the SWDGE cast-DMA; bf16 weights take the
    plain HWDGE path."""
    S, K, M = w_ap.shape
    shape = ShapeInfo(pdims=((P, K // P),) * S, fdims=(M,))
    w_ts = [w_ap[s].rearrange("(po pi) m -> pi po m", pi=P) for s in range(S)]
    is_bf16 = w_ap.dtype == dt.bfloat16

    def prod(nc_, md):
        t = pool.tile([P, md.k_subtiles, md.m_tile], dt.bfloat16, tag=tagname)
        eng = nc_.sync if is_bf16 else nc_.gpsimd
        eng.dma_start(
            t[:],
            w_ts[md.k_batch_idx][
                :, ts(md.k_tile_idx, md.k_subtiles),
                ds(md.m_tile_idx * md.m_tile, md.m_tile)
            ],
        )
        return t

    return prod, shape


def _w_producer_mbatched(pool, w_ap, tagname):
    """M-batched producer over [S, K, M] weights: fdims=(M,)*S, so one
    composable call covers all S experts' layer-1 matmuls (shared kxn)."""
    S, K, M = w_ap.shape
    shape = ShapeInfo(pdims=((P, K // P),), fdims=(M,) * S)
    w_ts = [w_ap[s].rearrange("(po pi) m -> pi po m", pi=P) for s in range(S)]
    is_bf16 = w_ap.dtype == dt.bfloat16

    def prod(nc_, md):
        t = pool.tile([P, md.k_subtiles, md.m_tile], dt.bfloat16, tag=tagname)
        eng = nc_.sync if is_bf16 else nc_.gpsimd
        eng.dma_start(
            t[:],
            w_ts[md.m_batch_idx][
                :, ts(md.k_tile_idx, md.k_subtiles),
                ds(md.m_tile_idx * md.m_tile, md.m_tile)
            ],
        )
        return t

    return prod, shape


def build_pr(cfg: Cfg, debug: bool = False, loop_n: int = 0):
    """Router-only phase: top-2 indices/weights from f32 x. No x cast (the
    host casts x to bf16 for the gather and the shared-expert phase)."""
    nc = bacc.Bacc("TRN2", target_bir_lowering=False, debug=debug)
    f32 = dt.float32
    xT = nc.dram_tensor("xT", [cfg.D, cfg.TPC], f32, kind="ExternalInput").ap()
    rw = nc.dram_tensor("rw", [cfg.D, cfg.E], f32, kind="ExternalInput").ap()
    ridx = nc.dram_tensor("ridx", [cfg.TPC, 8], dt.uint32, kind="ExternalOutput").ap()
    rwts = nc.dram_tensor("rwts", [cfg.TPC, 8], f32, kind="ExternalOutput").ap()

    x_po = cfg.D // P
    PO_CH = 4
    NPO = x_po // PO_CH
    NG = cfg.TPC // P

    with tile.TileContext(nc) as tc, _maybe_loop(tc, loop_n), ExitStack() as ctx:
        const = ctx.enter_context(tc.tile_pool(name="const", bufs=1))
        rw_sb = const.tile([P, x_po, cfg.E], f32)
        nc.sync.dma_start(rw_sb[:], rw.rearrange("(po pi) e -> pi po e", pi=P))
        lg_acc = const.tile([P, NG, cfg.E], f32)
        xsrc = ctx.enter_context(tc.tile_pool(name="xsrc", bufs=2))
        rps = ctx.enter_context(tc.tile_pool(name="rpsum", bufs=4, space="PSUM"))
        rsb = ctx.enter_context(tc.tile_pool(name="rsb", bufs=3))
        xT_t = _rearr2(xT)

        for po8 in range(NPO):
            xt = xsrc.tile([P, PO_CH, cfg.TPC], f32, tag="xt")
            nc.sync.dma_start(xt[:], xT_t[:, ts(po8, PO_CH), :])
            for tt in range(NG):
                ps = rps.tile([P, cfg.E], f32, tag="rp")
                for pp in range(PO_CH):
                    nc.tensor.matmul(
                        ps[:], xt[:, pp, ts(tt, P)],
                        rw_sb[:, po8 * PO_CH + pp, :],
                        start=(pp == 0), stop=(pp == PO_CH - 1),
                    )
                if po8 == 0:
                    nc.vector.tensor_copy(lg_acc[:, tt, :], ps[:])
                else:
                    nc.vector.tensor_add(lg_acc[:, tt, :], lg_acc[:, tt, :], ps[:])

        for tt in range(NG):
            t0 = tt * P
            lg = lg_acc[:, tt, :]
            mx = rsb.tile([P, 8], f32, tag="mx")
            nc.vector.max(mx[:], lg)
            ix = rsb.tile([P, 8], dt.uint32, tag="ix")
            nc.vector.max_index(ix[:], mx[:], lg)
            nm = rsb.tile([P, 1], f32, tag="nm")
            nc.vector.tensor_scalar_mul(nm[:], mx[:, 0:1], -1.0)
            ex = rsb.tile([P, cfg.E], f32, tag="ex")
            zz = rsb.tile([P, 1], f32, tag="zz")
            nc.scalar.activation(ex[:], lg, ActFn.Exp, bias=nm[:], accum_out=zz[:])
            rz = rsb.tile([P, 1], f32, tag="rz")
            nc.vector.reciprocal(rz[:], zz[:])
            wv = rsb.tile([P, 8], f32, tag="wv")
            nc.scalar.activation(wv[:], mx[:], ActFn.Exp, bias=nm[:])
            nc.vector.tensor_scalar_mul(wv[:], wv[:], rz[:])
            nc.sync.dma_start(ridx[ds(t0, P), :], ix[:])
            nc.sync.dma_start(rwts[ds(t0, P), :], wv[:])
    nc.compile()
    return nc


def build_p3s(cfg: Cfg, debug: bool = False, loop_n: int = 0, has_b2: bool = False):
    """Shared experts + combine: out = sharedFFN(x) + ya + yb.

    h is kept in SBUF as per-(s, n-tile) tiles so layer 2 pipelines with
    layer 1 at tile granularity (no DRAM roundtrip, no coarse-dep stall).
    """
    nc = bacc.Bacc("TRN2", target_bir_lowering=False, debug=debug)
    f32 = dt.float32
    bf16 = dt.bfloat16
    xbfT = nc.dram_tensor("xbfT", [cfg.D, cfg.TPC], dt.bfloat16, kind="ExternalInput").ap()
    sw1 = nc.dram_tensor("sw1", [cfg.NSH, cfg.D, cfg.DS], bf16, kind="ExternalInput").ap()
    sb1 = nc.dram_tensor("sb1", [cfg.NSH, cfg.DS], f32, kind="ExternalInput").ap()
    sw2 = nc.dram_tensor("sw2", [cfg.NSH, cfg.DS, cfg.D], bf16, kind="ExternalInput").ap()
    sb2 = nc.dram_tensor("sb2", [cfg.NSH, cfg.D], f32, kind="ExternalInput").ap()
    yaT = nc.dram_tensor("yaT", [cfg.D, cfg.TPC], dt.bfloat16, kind="ExternalInput").ap()
    ybT = nc.dram_tensor("ybT", [cfg.D, cfg.TPC], dt.bfloat16, kind="ExternalInput").ap()
    outT = nc.dram_tensor("outT", [cfg.D, cfg.TPC], bf16, kind="ExternalOutput").ap()

    x_po = cfg.D // P
    ds_po = cfg.DS // P
    NT = cfg.TPC // 512  # n tiles

    with tile.TileContext(nc) as tc, _maybe_loop(tc, loop_n), ExitStack() as ctx:
        const = ctx.enter_context(tc.tile_pool(name="const", bufs=1))
        b1_sb = const.tile([P, cfg.NSH, ds_po], f32)
        nc.sync.dma_start(b1_sb[:], sb1.rearrange("s (po pi) -> pi s po", pi=P))
        b2_sb = const.tile([P, cfg.NSH, x_po], f32)
        nc.sync.dma_start(b2_sb[:], sb2.rearrange("s (po pi) -> pi s po", pi=P))
        b2sum = const.tile([P, x_po], f32)
        nc.vector.tensor_add(b2sum[:], b2_sb[:, 0], b2_sb[:, 1])

        hpool = ctx.enter_context(tc.tile_pool(name="hp", bufs=1))
        h_tiles = [
            [
                hpool.tile([P, ds_po, 512], dt.bfloat16,
                           tag=f"h{s}_{n}", name=f"h{s}_{n}")
                for n in range(NT)
            ]
            for s in range(cfg.NSH)
        ]

        # x cached whole in SBUF: the composable would otherwise re-DMA each
        # kxn tile once per m_outer (4x24MB of reads for this shape).
        xsb = const.tile([P, x_po, cfg.TPC], bf16)
        nc.sync.dma_start(xsb[:], _rearr2(xbfT))

        with ExitStack() as c2:
            mpool = c2.enter_context(tc.tile_pool(name="l1m", bufs=3))
            kxm_prod, kxm_shape = _w_producer_mbatched(mpool, sw1, "sw1")
            kxn_shape = ShapeInfo(pdims=((P, x_po),), fdims=(cfg.TPC,))

            def x_kxn_producer(nc_, md):
                return xsb[:, ts(md.k_tile_idx, md.k_subtiles),
                           ts(md.n_tile_idx, md.n_tile)]

            kxn_prod = x_kxn_producer

            def l1_reducer(nc_, psum, sbuf, md):
                ko = (md.m_tile_idx * md.m_tile + md.m_subtile_idx * P) // P
                nc_.scalar.activation(
                    sbuf[:], psum[:], ActFn.Silu,
                    bias=b1_sb[:, md.m_batch_idx, ko:ko + 1]
                )

            def h_producer(nc_, md):
                return h_tiles[md.m_batch_idx][md.n_tile_idx][
                    :, ds(md.m_tile_idx * (md.m_tile // P), md.m_tile // P), :
                ]

            composable_matmul_tile_kernel(
                tc=tc,
                kxm_shape=kxm_shape,
                kxn_shape=kxn_shape,
                output_type=None,
                kxm_producer=kxm_prod,
                kxn_producer=kxn_prod,
                mxn_consumer=lambda nc_, sbuf, md: None,
                mxn_subtile_reducer=l1_reducer,
                mxn_subtile_producer=h_producer,
                psum_n_bufs=2,
                MAX_K_TILE_SIZE=1024,
            )

        with ExitStack() as c2:
            mpool = c2.enter_context(tc.tile_pool(name="l2m", bufs=4))
            apool = c2.enter_context(tc.tile_pool(name="addp", bufs=3))
            kxm_prod, kxm_shape = _w_producer_batched(mpool, sw2, "sw2")
            kxn_shape = ShapeInfo(pdims=((P, ds_po),) * cfg.NSH, fdims=(cfg.TPC,))

            def h_kxn_producer(nc_, md):
                return h_tiles[md.k_batch_idx][md.n_tile_idx][
                    :, ts(md.k_tile_idx, md.k_subtiles), :
                ]

            def l2_reducer(nc_, psum, sbuf, md):
                do = md.m_tile_idx * (md.m_tile // P) + md.m_subtile_idx
                if has_b2:
                    nc_.vector.tensor_scalar_add(sbuf[:], psum[:], b2sum[:, do:do + 1])
                else:
                    nc_.vector.tensor_copy(sbuf[:], psum[:])

            base_consumer = dma_to_dram_mxn(outT)
            yaT_t, ybT_t = _rearr2(yaT), _rearr2(ybT)

            def combine_consumer(nc_, sbuf, md):
                po0 = md.m_tile_idx * (md.m_tile // P)
                nsub = md.m_tile // P
                nsl = ds(md.n_tile_idx * md.n_tile, md.n_tile)
                ya_t = apool.tile([P, nsub, md.n_tile], dt.bfloat16, tag="ya")
                nc_.sync.dma_start(ya_t[:], yaT_t[:, ds(po0, nsub), nsl])
                yb_t = apool.tile([P, nsub, md.n_tile], dt.bfloat16, tag="yb")
                nc_.sync.dma_start(yb_t[:], ybT_t[:, ds(po0, nsub), nsl])
                nc_.vector.tensor_add(sbuf[:], sbuf[:], ya_t[:])
                nc_.vector.tensor_add(sbuf[:], sbuf[:], yb_t[:])
                base_consumer(nc_, sbuf, md)

            composable_matmul_tile_kernel(
                tc=tc,
                kxm_shape=kxm_shape,
                kxn_shape=kxn_shape,
                output_type=dt.bfloat16,
                kxm_producer=kxm_prod,
                kxn_producer=h_kxn_producer,
                mxn_consumer=combine_consumer,
                mxn_subtile_reducer=l2_reducer,
                psum_n_bufs=2,
                MAX_K_TILE_SIZE=1024,
            )
    nc.compile()
    return nc


# --------------------------------------------------------------------------
# Phase 2: routed experts (expert-parallel, capacity padded)
# --------------------------------------------------------------------------

def _w_producer(pool, w_ap, tagname):
    """kxm producer streaming a [K, M] weight from DRAM as bf16. f32 source
    uses the SWDGE cast-DMA; bf16 source takes the plain HWDGE path."""
    K, M = w_ap.shape
    shape = ShapeInfo(pdims=((P, K // P),), fdims=(M,))
    w_t = w_ap.rearrange("(po pi) m -> pi po m", pi=P)
    is_bf16 = w_ap.dtype == dt.bfloat16

    def prod(nc_, md):
        t = pool.tile(
            [P, md.k_subtiles, md.m_tile], dt.bfloat16, tag=tagname
        )
        eng = nc_.sync if is_bf16 else nc_.gpsimd
        eng.dma_start(
            t[:],
            w_t[
                :, ts(md.k_tile_idx, md.k_subtiles),
                ds(md.m_tile_idx * md.m_tile, md.m_tile)
            ],
        )
        return t

    return prod, shape


def build_p2(cfg: Cfg, debug: bool = False, loop_n: int = 0, has_b2: bool = False):
    nc = bacc.Bacc("TRN2", target_bir_lowering=False, debug=debug)
    f32 = dt.float32
    W = cfg.EPC * cfg.CAP
    xgT = nc.dram_tensor("xgT", [cfg.D, W], dt.bfloat16, kind="ExternalInput").ap()
    ew1 = nc.dram_tensor("ew1", [cfg.EPC, cfg.D, cfg.DR], dt.bfloat16, kind="ExternalInput").ap()
    eb1 = nc.dram_tensor("eb1", [cfg.EPC, cfg.DR], f32, kind="ExternalInput").ap()
    ew2 = nc.dram_tensor("ew2", [cfg.EPC, cfg.DR, cfg.D], dt.bfloat16, kind="ExternalInput").ap()
    eb2 = nc.dram_tensor("eb2", [cfg.EPC, cfg.D], f32, kind="ExternalInput").ap()
    cw = nc.dram_tensor("cw", [cfg.EPC, cfg.CAPP], f32, kind="ExternalInput").ap()
    ygT = nc.dram_tensor("ygT", [cfg.D, W], dt.bfloat16, kind="ExternalOutput").ap()

    x_po = cfg.D // P
    dr_po = cfg.DR // P

    with tile.TileContext(nc) as tc, _maybe_loop(tc, loop_n), ExitStack() as ctx:
        const = ctx.enter_context(tc.tile_pool(name="const", bufs=1))
        b1_sb = const.tile([P, cfg.EPC, dr_po], f32)
        nc.sync.dma_start(b1_sb[:], eb1.rearrange("e (po pi) -> pi e po", pi=P))
        if has_b2:
            b2_sb = const.tile([P, cfg.EPC, x_po], f32)
            nc.sync.dma_start(b2_sb[:], eb2.rearrange("e (po pi) -> pi e po", pi=P))
        cwrep = const.tile([P, cfg.EPC, cfg.CAPP], f32)
        nc.sync.dma_start(
            cwrep[:],
            cw.rearrange("e c -> (e c)")[None].to_broadcast((P, cfg.EPC * cfg.CAPP)),
        )

        hg_shape = ShapeInfo(pdims=((P, dr_po),), fdims=(cfg.CAP,))
        hg_pool = ctx.enter_context(tc.tile_pool(name="hg", bufs=1))
        hg_tiles = [
            [
                hg_pool.tile([P, dr_po, cfg.n_tile], dt.bfloat16,
                             tag=f"hg{e}_{n}", name=f"hg{e}_{n}")
                for n in range(cfg.n_tiles)
            ]
            for e in range(cfg.EPC)
        ]

        # layer 1 for all experts first (keeps the PE stream dense)
        for e in range(cfg.EPC):
            with ExitStack() as c2:
                mpool = c2.enter_context(tc.tile_pool(name=f"p2m{e}", bufs=3))
                npool = c2.enter_context(tc.tile_pool(name=f"p2n{e}", bufs=5))
                kxm_prod, kxm_shape = _w_producer(mpool, ew1[e], f"w1_{e}")
                kxn_prod, kxn_shape = dma_from_dram_kxn(
                    npool, xgT[:, ds(e * cfg.CAP, cfg.CAP)]
                )

                def l1_reducer(nc_, psum, sbuf, md, e=e):
                    ko = (md.m_tile_idx * md.m_tile + md.m_subtile_idx * P) // P
                    nc_.scalar.activation(
                        sbuf[:], psum[:], ActFn.Silu, bias=b1_sb[:, e, ko:ko + 1]
                    )

                def hg_producer(nc_, md, e=e):
                    return hg_tiles[e][md.n_tile_idx][
                        :, ts(md.m_tile_idx, md.m_tile // P), :
                    ]

                composable_matmul_tile_kernel(
                    tc=tc,
                    kxm_shape=kxm_shape,
                    kxn_shape=kxn_shape,
                    output_type=None,
                    kxm_producer=kxm_prod,
                    kxn_producer=kxn_prod,
                    mxn_consumer=lambda nc_, sbuf, md: None,
                    mxn_subtile_reducer=l1_reducer,
                    mxn_subtile_producer=hg_producer,
                    psum_n_bufs=2,
                    MAX_K_TILE_SIZE=1024,
                )

        # layer 2 for all experts
        for e in range(cfg.EPC):
            with ExitStack() as c2:
                m2pool = c2.enter_context(tc.tile_pool(name=f"p2m2{e}", bufs=2))
                tpool = c2.enter_context(tc.tile_pool(name=f"p2t{e}", bufs=3))
                kxm2_prod, kxm2_shape = _w_producer(m2pool, ew2[e], f"w2_{e}")

                def hg_kxn_producer(nc_, md, e=e):
                    return hg_tiles[e][md.n_tile_idx][
                        :, ts(md.k_tile_idx, md.k_subtiles), :
                    ]

                def l2_reducer(nc_, psum, sbuf, md, e=e):
                    do = (md.m_tile_idx * md.m_tile + md.m_subtile_idx * P) // P
                    n0 = md.n_tile_idx * md.n_tile + md.n_subtile_idx * md.n_subtile
                    if has_b2:
                        stage = tpool.tile([P, md.n_subtile], dt.float32, tag="stage")
                        nc_.vector.tensor_scalar_add(
                            stage[:], psum[:], b2_sb[:, e, do:do + 1]
                        )
                        src = stage
                    else:
                        src = psum
                    nc_.vector.tensor_mul(
                        sbuf[:], src[:], cwrep[:, e, ds(n0, md.n_subtile)]
                    )

                composable_matmul_tile_kernel(
                    tc=tc,
                    kxm_shape=kxm2_shape,
                    kxn_shape=hg_shape,
                    output_type=dt.bfloat16,
                    kxm_producer=kxm2_prod,
                    kxn_producer=hg_kxn_producer,
                    mxn_consumer=dma_to_dram_mxn(ygT[:, ds(e * cfg.CAP, cfg.CAP)]),
                    mxn_subtile_reducer=l2_reducer,
                    psum_n_bufs=2,
                )

    nc.compile()
    return nc


def build_p2w(cfg: Cfg, debug: bool = False, loop_n: int = 0):
    """Weight-stationary routed-expert FFN (assumes zero b2).

    Both experts' w1/w2 are preloaded into SBUF (bf16), so the PE never
    waits on weight DMA. Tokens stream in n-groups of 1024 with double
    buffering; each loaded stationary block feeds 2 moving tiles. Per
    expert: L1 over all groups (h kept in SBUF), then L2 over all groups
    (pre-scaled by the combine weight). Long uninterrupted matmul chains
    keep the PE p-state ramped.
    """
    nc = bacc.Bacc("TRN2", target_bir_lowering=False, debug=debug)
    f32 = dt.float32
    bf16 = dt.bfloat16
    W = cfg.EPC * cfg.CAP
    xgT = nc.dram_tensor("xgT", [cfg.D, W], bf16, kind="ExternalInput").ap()
    ew1 = nc.dram_tensor("ew1", [cfg.EPC, cfg.D, cfg.DR], bf16, kind="ExternalInput").ap()
    eb1 = nc.dram_tensor("eb1", [cfg.EPC, cfg.DR], f32, kind="ExternalInput").ap()
    ew2 = nc.dram_tensor("ew2", [cfg.EPC, cfg.DR, cfg.D], bf16, kind="ExternalInput").ap()
    cw = nc.dram_tensor("cw", [cfg.EPC, cfg.CAPP], f32, kind="ExternalInput").ap()
    ygT = nc.dram_tensor("ygT", [cfg.D, W], bf16, kind="ExternalOutput").ap()

    x_po = cfg.D // P
    dr_po = cfg.DR // P
    NG = 1024
    groups = []
    g0 = 0
    while g0 < cfg.CAP:
        gn = min(NG, cfg.CAP - g0)
        groups.append((g0, gn))
        g0 += gn

    with tile.TileContext(nc) as tc, _maybe_loop(tc, loop_n), ExitStack() as ctx:
        const = ctx.enter_context(tc.tile_pool(name="const", bufs=1))
        b1_sb = const.tile([P, cfg.EPC, dr_po], f32)
        nc.sync.dma_start(b1_sb[:], eb1.rearrange("e (po pi) -> pi e po", pi=P))
        cwrep = const.tile([P, cfg.EPC, cfg.CAPP], f32)
        nc.sync.dma_start(
            cwrep[:],
            cw.rearrange("e c -> (e c)")[None].to_broadcast((P, cfg.EPC * cfg.CAPP)),
        )
        w1_sb = const.tile([P, cfg.EPC, x_po, cfg.DR], bf16)
        w2_sb = const.tile([P, cfg.EPC, dr_po, cfg.D], bf16)
        for e in range(cfg.EPC):
            nc.sync.dma_start(
                w1_sb[:, e], ew1[e].rearrange("(po pi) m -> pi po m", pi=P))
            nc.sync.dma_start(
                w2_sb[:, e], ew2[e].rearrange("(po pi) m -> pi po m", pi=P))
        hg = [const.tile([P, dr_po, cfg.CAP], bf16, name=f"hg{e}", tag=f"hg{e}")
              for e in range(cfg.EPC)]

        xpool = ctx.enter_context(tc.tile_pool(name="xg", bufs=2))
        l1ps = ctx.enter_context(tc.tile_pool(name="l1ps", bufs=2, space="PSUM"))
        l2ps = ctx.enter_context(tc.tile_pool(name="l2ps", bufs=2, space="PSUM"))
        stage = ctx.enter_context(tc.tile_pool(name="stage", bufs=6))
        xgT_t = _rearr2(xgT)
        ygT_t = _rearr2(ygT)

        for e in range(cfg.EPC):
            for g0, gn in groups:
                xg = xpool.tile([P, x_po, gn], bf16, tag=f"xg{gn}")
                nc.sync.dma_start(xg[:], xgT_t[:, :, ds(e * cfg.CAP + g0, gn)])
                nts = [(nt * 512, min(512, gn - nt * 512))
                       for nt in range(-(-gn // 512))]
                for m in range(dr_po):
                    ps = [l1ps.tile([P, 512], f32, tag=f"l1p{i}", name=f"l1p{i}")
                          for i in range(len(nts))]
                    for k in range(x_po):
                        for i, (n0, nw) in enumerate(nts):
                            nc.tensor.matmul(
                                ps[i][:, :nw], w1_sb[:, e, k, ts(m, P)],
                                xg[:, k, ds(n0, nw)],
                                start=(k == 0), stop=(k == x_po - 1),
                            )
                    for i, (n0, nw) in enumerate(nts):
                        nc.scalar.activation(
                            hg[e][:, m, ds(g0 + n0, nw)], ps[i][:, :nw],
                            ActFn.Silu, bias=b1_sb[:, e, m:m + 1])
            for g0, gn in groups:
                nts = [(nt * 512, min(512, gn - nt * 512))
                       for nt in range(-(-gn // 512))]
                for mo in range(x_po):
                    ps2 = [l2ps.tile([P, 512], f32, tag=f"l2p{i}", name=f"l2p{i}")
                           for i in range(len(nts))]
                    for k in range(dr_po):
                        for i, (n0, nw) in enumerate(nts):
                            nc.tensor.matmul(
                                ps2[i][:, :nw], w2_sb[:, e, k, ts(mo, P)],
                                hg[e][:, k, ds(g0 + n0, nw)],
                                start=(k == 0), stop=(k == dr_po - 1),
                            )
                    for i, (n0, nw) in enumerate(nts):
                        st = stage.tile([P, 512], bf16, tag="st")
                        nc.vector.tensor_mul(
                            st[:, :nw], ps2[i][:, :nw],
                            cwrep[:, e, ds(g0 + n0, nw)])
                        nc.sync.dma_start(
                            ygT_t[:, mo, ds(e * cfg.CAP + g0 + n0, nw)],
                            st[:, :nw])
    nc.compile()
    return nc


# --------------------------------------------------------------------------
# Fused pipeline: p1f (router + shared L1), p2f (fp8 routed), p3f (L2+combine)
# --------------------------------------------------------------------------

def build_p1f(cfg: Cfg, debug: bool = False, loop_n: int = 0):
    """Router (true-f32 for exact top-2) + shared-expert layer 1.

    The router's x read and PE work hide under the shared-L1 matmul stream;
    x is cast to bf16 on the DVE from the same f32 tiles the router uses.
    h = silu(x @ sw1 + b1) goes to DRAM in bf16 for p3f.
    """
    nc = bacc.Bacc("TRN2", target_bir_lowering=False, debug=debug)
    f32 = dt.float32
    bf16 = dt.bfloat16
    xT = nc.dram_tensor("xT", [cfg.D, cfg.TPC], f32, kind="ExternalInput").ap()
    rw = nc.dram_tensor("rw", [cfg.D, cfg.E], f32, kind="ExternalInput").ap()
    sw1 = nc.dram_tensor("sw1", [cfg.NSH, cfg.D, cfg.DS], bf16, kind="ExternalInput").ap()
    sb1 = nc.dram_tensor("sb1", [cfg.NSH, cfg.DS], f32, kind="ExternalInput").ap()
    ridx = nc.dram_tensor("ridx", [cfg.TPC, 8], dt.uint32, kind="ExternalOutput").ap()
    rwts = nc.dram_tensor("rwts", [cfg.TPC, 8], f32, kind="ExternalOutput").ap()
    hT = nc.dram_tensor("hT", [cfg.NSH, cfg.DS, cfg.TPC], bf16, kind="ExternalOutput").ap()

    x_po = cfg.D // P
    ds_po = cfg.DS // P
    PO_CH = 2
    NPO = x_po // PO_CH
    NG = cfg.TPC // P
    NCH = cfg.TPC // 512  # 512-token chunks

    with tile.TileContext(nc) as tc, _maybe_loop(tc, loop_n), ExitStack() as ctx:
        const = ctx.enter_context(tc.tile_pool(name="const", bufs=1))
        rw_sb = const.tile([P, x_po, cfg.E], f32)
        nc.sync.dma_start(rw_sb[:], rw.rearrange("(po pi) e -> pi po e", pi=P))
        b1_sb = const.tile([P, cfg.NSH, ds_po], f32)
        nc.sync.dma_start(b1_sb[:], sb1.rearrange("s (po pi) -> pi s po", pi=P))
        lg_acc = const.tile([P, NG, cfg.E], f32)
        xbf = const.tile([P, x_po, cfg.TPC], bf16)
        w1_sb = const.tile([P, cfg.NSH, x_po, cfg.DS], bf16)

        xsrc = ctx.enter_context(tc.tile_pool(name="xsrc", bufs=2))
        rps = ctx.enter_context(tc.tile_pool(name="rpsum", bufs=2, space="PSUM"))
        rsb = ctx.enter_context(tc.tile_pool(name="rsb", bufs=3))
        l1ps = ctx.enter_context(tc.tile_pool(name="l1ps", bufs=2, space="PSUM"))
        hstage = ctx.enter_context(tc.tile_pool(name="hst", bufs=4))
        xT_t = _rearr2(xT)

        # ---- router matmuls + bf16 cast of x (PO_CH po-slices at a time) ----
        for po8 in range(NPO):
            xt = xsrc.tile([P, PO_CH, cfg.TPC], f32, tag="xt")
            nc.sync.dma_start(xt[:], xT_t[:, ts(po8, PO_CH), :])
            nc.vector.tensor_copy(xbf[:, ts(po8, PO_CH), :], xt[:])
            for tt in range(NG):
                ps = rps.tile([P, cfg.E], f32, tag="rp")
                for pp in range(PO_CH):
                    nc.tensor.matmul(
                        ps[:], xt[:, pp, ts(tt, P)],
                        rw_sb[:, po8 * PO_CH + pp, :],
                        start=(pp == 0), stop=(pp == PO_CH - 1),
                    )
                if po8 == 0:
                    nc.vector.tensor_copy(lg_acc[:, tt, :], ps[:])
                else:
                    nc.vector.tensor_add(lg_acc[:, tt, :], lg_acc[:, tt, :], ps[:])

        # w1 queued after x so the router never waits on it
        for s in range(cfg.NSH):
            nc.sync.dma_start(
                w1_sb[:, s], sw1[s].rearrange("(po pi) m -> pi po m", pi=P))

        # ---- top-2 softmax ----
        for tt in range(NG):
            t0 = tt * P
            lg = lg_acc[:, tt, :]
            mx = rsb.tile([P, 8], f32, tag="mx")
            nc.vector.max(mx[:], lg)
            ix = rsb.tile([P, 8], dt.uint32, tag="ix")
            nc.vector.max_index(ix[:], mx[:], lg)
            nm = rsb.tile([P, 1], f32, tag="nm")
            nc.vector.tensor_scalar_mul(nm[:], mx[:, 0:1], -1.0)
            ex = rsb.tile([P, cfg.E], f32, tag="ex")
            zz = rsb.tile([P, 1], f32, tag="zz")
            nc.scalar.activation(ex[:], lg, ActFn.Exp, bias=nm[:], accum_out=zz[:])
            rz = rsb.tile([P, 1], f32, tag="rz")
            nc.vector.reciprocal(rz[:], zz[:])
            wv = rsb.tile([P, 8], f32, tag="wv")
            nc.scalar.activation(wv[:], mx[:], ActFn.Exp, bias=nm[:])
            nc.vector.tensor_scalar_mul(wv[:], wv[:], rz[:])
            nc.sync.dma_start(ridx[ds(t0, P), :], ix[:])
            nc.sync.dma_start(rwts[ds(t0, P), :], wv[:])

        # ---- shared layer 1 (weight-stationary, 2-chunk LDW amortization) ----
        hT_t = [hT[s].rearrange("(po pi) t -> pi po t", pi=P)
                for s in range(cfg.NSH)]
        for e in range(cfg.NSH):
            for m in range(ds_po):
                for cp in range(NCH // 2):
                    ps2 = [l1ps.tile([P, 512], f32, tag=f"l1p{i}",
                                     name=f"l1p{i}") for i in range(2)]
                    for k in range(x_po):
                        for i in range(2):
                            nc.tensor.matmul(
                                ps2[i][:], w1_sb[:, e, k, ts(m, P)],
                                xbf[:, k, ds((2 * cp + i) * 512, 512)],
                                start=(k == 0), stop=(k == x_po - 1),
                            )
                    for i in range(2):
                        hst = hstage.tile([P, 512], bf16, tag="hst")
                        nc.scalar.activation(
                            hst[:], ps2[i][:], ActFn.Silu,
                            bias=b1_sb[:, e, m:m + 1])
                        nc.sync.dma_start(
                            hT_t[e][:, m, ds((2 * cp + i) * 512, 512)], hst[:])
    nc.compile()
    return nc


def build_p1g(cfg: Cfg, debug: bool = False, loop_n: int = 0):
    """Router (3-term bf16 split, exact top-2 to ~1e-5) + shared layer 1.

    logits = x_hi·w_hi + x_lo·w_hi + x_hi·w_lo with x_hi/x_lo the bf16
    hi/lo split of f32 x (computed on host).  Router weights are the
    stationary operand so each 512-token group costs 48 N=512 matmuls;
    raw logits go to DRAM and the host does top-2 + softmax exactly.
    h = silu(x @ sw1 + b1) -> DRAM bf16 for p3f.
    """
    nc = bacc.Bacc("TRN2", target_bir_lowering=False, debug=debug)
    f32 = dt.float32
    bf16 = dt.bfloat16
    xbfT = nc.dram_tensor("xbfT", [cfg.D, cfg.TPC], bf16, kind="ExternalInput").ap()
    xloT = nc.dram_tensor("xloT", [cfg.D, cfg.TPC], bf16, kind="ExternalInput").ap()
    rwh = nc.dram_tensor("rwh", [cfg.D, cfg.E], bf16, kind="ExternalInput").ap()
    rwl = nc.dram_tensor("rwl", [cfg.D, cfg.E], bf16, kind="ExternalInput").ap()
    sw1 = nc.dram_tensor("sw1", [cfg.NSH, cfg.D, cfg.DS], bf16, kind="ExternalInput").ap()
    sb1 = nc.dram_tensor("sb1", [cfg.NSH, cfg.DS], f32, kind="ExternalInput").ap()
    lgT = nc.dram_tensor("lgT", [cfg.E, cfg.TPC], f32, kind="ExternalOutput").ap()
    hT = nc.dram_tensor("hT", [cfg.NSH, cfg.DS, cfg.TPC], bf16, kind="ExternalOutput").ap()

    x_po = cfg.D // P
    ds_po = cfg.DS // P
    NG4 = cfg.TPC // 512

    with tile.TileContext(nc) as tc, _maybe_loop(tc, loop_n), ExitStack() as ctx:
        const = ctx.enter_context(tc.tile_pool(name="const", bufs=1))
        rwh_sb = const.tile([P, x_po, cfg.E], bf16)
        rwl_sb = const.tile([P, x_po, cfg.E], bf16)
        b1_sb = const.tile([P, cfg.NSH, ds_po], f32)
        nc.sync.dma_start(b1_sb[:], sb1.rearrange("s (po pi) -> pi s po", pi=P))
        xbf = const.tile([P, x_po, cfg.TPC], bf16)
        w1_sb = const.tile([P, cfg.NSH, x_po, cfg.DS], bf16)

        nc.sync.dma_start(rwh_sb[:], rwh.rearrange("(po pi) e -> pi po e", pi=P))
        nc.sync.dma_start(rwl_sb[:], rwl.rearrange("(po pi) e -> pi po e", pi=P))
        xlop = ctx.enter_context(tc.tile_pool(name="xlo", bufs=1))
        lsb = ctx.enter_context(tc.tile_pool(name="lsb", bufs=3))
        xloT_t = _rearr2(xloT)
        # DMA order: 3 xlo groups (small, unblocks router B-chains), the full
        # xbf, the last xlo group (reusing buffer 0 — its B-chain runs first
        # in group 0 so the buffer frees early), then w1 (L1-only).
        xlo_tiles = [
            xlop.tile([P, x_po, 512], bf16, name=f"xlo{g}", tag=f"xlo{g}")
            for g in range(NG4 - 1)
        ]
        xbfT_t = _rearr2(xbfT)
        # interleave xlo/xbf group slices so router group g is ready at
        # ~6*(g+1) us instead of after the whole 8.4MB xbf load
        for g in range(NG4 - 1):
            nc.sync.dma_start(xlo_tiles[g][:], xloT_t[:, :, ts(g, 512)])
            nc.sync.dma_start(xbf[:, :, ts(g, 512)], xbfT_t[:, :, ts(g, 512)])
        nc.sync.dma_start(xbf[:, :, ts(NG4 - 1, 512)],
                          xbfT_t[:, :, ts(NG4 - 1, 512)])
        xlo_last = xlop.tile([P, x_po, 512], bf16, name="xlo0", tag="xlo0")
        nc.sync.dma_start(xlo_last[:], xloT_t[:, :, ts(NG4 - 1, 512)])
        xlo_tiles.append(xlo_last)
        for s in range(cfg.NSH):
            nc.sync.dma_start(
                w1_sb[:, s], sw1[s].rearrange("(po pi) m -> pi po m", pi=P))

        with ExitStack() as c2:
            rps = c2.enter_context(tc.tile_pool(name="rps", bufs=2, space="PSUM"))
            for g in range(NG4):
                gsl = ts(g, 512)
                # one 48-matmul accumulation group: B + H + L terms
                psR = rps.tile([cfg.E, 512], f32, tag="psR", name="psR")
                for po in range(x_po):
                    nc.tensor.matmul(psR[:], rwh_sb[:, po, :],
                                     xlo_tiles[g][:, po, :],
                                     start=(po == 0), stop=False)
                for po in range(x_po):
                    nc.tensor.matmul(psR[:], rwh_sb[:, po, :], xbf[:, po, gsl],
                                     start=False, stop=False)
                for po in range(x_po):
                    nc.tensor.matmul(psR[:], rwl_sb[:, po, :], xbf[:, po, gsl],
                                     start=False, stop=(po == x_po - 1))
                lg = lsb.tile([cfg.E, 512], f32, tag="lg")
                nc.vector.tensor_copy(lg[:], psR[:])
                nc.sync.dma_start(lgT[:, gsl], lg[:])

        # ---- shared layer 1 (stationary shared across 4 token chunks) ----
        l1ps = ctx.enter_context(tc.tile_pool(name="l1ps", bufs=2, space="PSUM"))
        hstage = ctx.enter_context(tc.tile_pool(name="hst", bufs=4))
        hT_t = [hT[s].rearrange("(po pi) t -> pi po t", pi=P)
                for s in range(cfg.NSH)]
        for e in range(cfg.NSH):
            for m in range(ds_po):
                ps4 = [l1ps.tile([P, 512], f32, tag=f"l1p{i}",
                                 name=f"l1p{i}") for i in range(NG4)]
                for k in range(x_po):
                    for i in range(NG4):
                        nc.tensor.matmul(
                            ps4[i][:], w1_sb[:, e, k, ts(m, P)],
                            xbf[:, k, ts(i, 512)],
                            start=(k == 0), stop=(k == x_po - 1),
                        )
                for i in range(NG4):
                    hst = hstage.tile([P, 512], bf16, tag="hst")
                    nc.scalar.activation(
                        hst[:], ps4[i][:], ActFn.Silu,
                        bias=b1_sb[:, e, m:m + 1])
                    nc.sync.dma_start(hT_t[e][:, m, ts(i, 512)], hst[:])
    nc.compile()
    return nc


def build_p2f(cfg: Cfg, debug: bool = False, loop_n: int = 0):
    """Routed-expert FFN in fp8 (DoubleRow, ~1.44x PE) with per-slot caps.

    Weights arrive pre-scaled by 16 (fp8e4); layer-1 undoes the scale in the
    silu (scale=1/16), layer-2's 16 is folded into cw by the host. Assumes
    zero b2 (the host falls back to the bf16 p2/p2w path otherwise).
    """
    nc = bacc.Bacc("TRN2", target_bir_lowering=False, debug=debug)
    f32 = dt.float32
    bf16 = dt.bfloat16
    fp8 = dt.float8e4
    W = cfg.W2
    caps = (cfg.CAP_A, cfg.CAP_B)
    offs = (0, cfg.CAP_A)
    xgT = nc.dram_tensor("xgT", [cfg.D, W], fp8, kind="ExternalInput").ap()
    ew1 = nc.dram_tensor("ew1", [cfg.EPC, cfg.D, cfg.DR], fp8, kind="ExternalInput").ap()
    eb1 = nc.dram_tensor("eb1", [cfg.EPC, cfg.DR], f32, kind="ExternalInput").ap()
    ew2 = nc.dram_tensor("ew2", [cfg.EPC, cfg.DR, cfg.D], fp8, kind="ExternalInput").ap()
    cw = nc.dram_tensor("cw", [W], f32, kind="ExternalInput").ap()
    ygT = nc.dram_tensor("ygT", [cfg.D, W], bf16, kind="ExternalOutput").ap()

    x_po = cfg.D // P
    dr_po = cfg.DR // P
    DR_MODE = mybir.MatmulPerfMode.DoubleRow

    def chunks(cap):
        out, c0 = [], 0
        while c0 < cap:
            cn = min(512, cap - c0)
            out.append((c0, cn))
            c0 += cn
        return out

    with tile.TileContext(nc) as tc, _maybe_loop(tc, loop_n), ExitStack() as ctx:
        const = ctx.enter_context(tc.tile_pool(name="const", bufs=1))
        b1_sb = const.tile([P, cfg.EPC, dr_po], f32)
        nc.sync.dma_start(b1_sb[:], eb1.rearrange("e (po pi) -> pi e po", pi=P))
        cwrep = const.tile([P, W], f32)
        nc.sync.dma_start(cwrep[:], cw[None].to_broadcast((P, W)))
        w1_sb = const.tile([P, cfg.EPC, x_po, cfg.DR], fp8)
        w2_sb = const.tile([P, cfg.EPC, dr_po, cfg.D], fp8)
        hg = [const.tile([P, dr_po, caps[e]], fp8, name=f"hg{e}", tag=f"hg{e}")
              for e in range(cfg.EPC)]

        xpool = ctx.enter_context(tc.tile_pool(name="xg", bufs=3))
        l1ps = ctx.enter_context(tc.tile_pool(name="l1ps", bufs=3, space="PSUM"))
        l2ps = ctx.enter_context(tc.tile_pool(name="l2ps", bufs=4, space="PSUM"))
        stage = ctx.enter_context(tc.tile_pool(name="stage", bufs=6))
        xgT_t = _rearr2(xgT)
        ygT_t = _rearr2(ygT)

        for e in range(cfg.EPC):
            nc.sync.dma_start(
                w1_sb[:, e], ew1[e].rearrange("(po pi) m -> pi po m", pi=P))
        for e in range(cfg.EPC):
            nc.sync.dma_start(
                w2_sb[:, e], ew2[e].rearrange("(po pi) m -> pi po m", pi=P))

        # ---- layer 1 (DoubleRow: contract 256 rows per matmul) ----
        for e in range(cfg.EPC):
            for g0, gn in chunks(caps[e]):
                xg = xpool.tile([P, x_po, gn], fp8, tag=f"xg{gn}")
                nc.sync.dma_start(xg[:], xgT_t[:, :, ds(offs[e] + g0, gn)])
                for m in range(dr_po):
                    ps = l1ps.tile([P, 512], f32, tag="l1p")
                    for k2 in range(x_po // 2):
                        nc.tensor.matmul(
                            ps[:, :gn], w1_sb[:, e, ds(2 * k2, 2), ts(m, P)],
                            xg[:, ds(2 * k2, 2), :],
                            start=(k2 == 0), stop=(k2 == x_po // 2 - 1),
                            perf_mode=DR_MODE,
                        )
                    nc.scalar.activation(
                        hg[e][:, m, ds(g0, gn)], ps[:, :gn], ActFn.Silu,
                        bias=b1_sb[:, e, m:m + 1], scale=1.0 / 16.0)

        # ---- layer 2 (DoubleRow over DR; output pre-scaled by cw/16) ----
        for e in range(cfg.EPC):
            for g0, gn in chunks(caps[e]):
                for mo in range(x_po):
                    ps = l2ps.tile([P, 512], f32, tag="l2p")
                    for k2 in range(dr_po // 2):
                        nc.tensor.matmul(
                            ps[:, :gn], w2_sb[:, e, ds(2 * k2, 2), ts(mo, P)],
                            hg[e][:, ds(2 * k2, 2), ds(g0, gn)],
                            start=(k2 == 0), stop=(k2 == dr_po // 2 - 1),
                            perf_mode=DR_MODE,
                        )
                    st = stage.tile([P, 512], bf16, tag="st")
                    nc.vector.tensor_mul(
                        st[:, :gn], ps[:, :gn],
                        cwrep[:, ds(offs[e] + g0, gn)])
                    nc.sync.dma_start(
                        ygT_t[:, mo, ds(offs[e] + g0, gn)], st[:, :gn])
    nc.compile()
    return nc


def build_p3f(cfg: Cfg, debug: bool = False, loop_n: int = 0, has_b2: bool = False):
    """Shared layer 2 + combine, f32 accumulation and f32 output.

    out = h @ sw2 (+ b2sum) + ya + yb with the adds on the DVE reading the
    f32 PSUM directly; the only low-precision steps left are the bf16/fp8
    matmul operands themselves.
    """
    nc = bacc.Bacc("TRN2", target_bir_lowering=False, debug=debug)
    f32 = dt.float32
    bf16 = dt.bfloat16
    hT = nc.dram_tensor("hT", [cfg.NSH, cfg.DS, cfg.TPC], bf16, kind="ExternalInput").ap()
    sw2 = nc.dram_tensor("sw2", [cfg.NSH, cfg.DS, cfg.D], bf16, kind="ExternalInput").ap()
    sb2 = nc.dram_tensor("sb2", [cfg.NSH, cfg.D], f32, kind="ExternalInput").ap()
    yaT = nc.dram_tensor("yaT", [cfg.D, cfg.TPC], bf16, kind="ExternalInput").ap()
    ybT = nc.dram_tensor("ybT", [cfg.D, cfg.TPC], bf16, kind="ExternalInput").ap()
    outT = nc.dram_tensor("outT", [cfg.D, cfg.TPC], f32, kind="ExternalOutput").ap()

    x_po = cfg.D // P
    ds_po = cfg.DS // P
    NCH = cfg.TPC // 512

    with tile.TileContext(nc) as tc, _maybe_loop(tc, loop_n), ExitStack() as ctx:
        const = ctx.enter_context(tc.tile_pool(name="const", bufs=1))
        b2_sb = const.tile([P, cfg.NSH, x_po], f32)
        nc.sync.dma_start(b2_sb[:], sb2.rearrange("s (po pi) -> pi s po", pi=P))
        b2sum = const.tile([P, x_po], f32)
        nc.vector.tensor_add(b2sum[:], b2_sb[:, 0], b2_sb[:, 1])
        h_sb = const.tile([P, cfg.NSH, ds_po, cfg.TPC], bf16)
        w2_sb = const.tile([P, cfg.NSH, ds_po, cfg.D], bf16)
        # interleave (h, w2) k-slices so the first contraction chain can
        # start as soon as the first slices land
        for e in range(cfg.NSH):
            hT_e = hT[e].rearrange("(po pi) t -> pi po t", pi=P)
            w2_e = sw2[e].rearrange("(po pi) m -> pi po m", pi=P)
            for k in range(ds_po):
                nc.sync.dma_start(h_sb[:, e, k], hT_e[:, k])
                nc.sync.dma_start(w2_sb[:, e, k], w2_e[:, k])

        l2ps = ctx.enter_context(tc.tile_pool(name="l2ps", bufs=1, space="PSUM"))
        ypool = ctx.enter_context(tc.tile_pool(name="yp", bufs=2))
        stage = ctx.enter_context(tc.tile_pool(name="stage", bufs=3))
        yaT_t, ybT_t, outT_t = _rearr2(yaT), _rearr2(ybT), _rearr2(outT)

        # Software pipeline: expert-1 chains lag expert-0 by LAG mo-blocks so
        # the PE streams e0 work while e1's h/w2 slices are still loading.
        # Stationaries are shared across all 4 token chunks (LDW/4).
        LAG = 2

        def emit_e0(mo):
            yts = []
            for i in range(NCH):
                ya_t = ypool.tile([P, 512], bf16, tag=f"ya{i}")
                nc.sync.dma_start(ya_t[:], yaT_t[:, mo, ts(i, 512)])
                yb_t = ypool.tile([P, 512], bf16, tag=f"yb{i}")
                nc.sync.dma_start(yb_t[:], ybT_t[:, mo, ts(i, 512)])
                yts.append((ya_t, yb_t))
            ps0 = [l2ps.tile([P, 512], f32, tag=f"l2a{i}", name=f"l2a{i}")
                   for i in range(NCH)]
            for k in range(ds_po):
                for i in range(NCH):
                    nc.tensor.matmul(
                        ps0[i][:], w2_sb[:, 0, k, ts(mo, P)],
                        h_sb[:, 0, k, ts(i, 512)],
                        start=(k == 0), stop=(k == ds_po - 1),
                    )
            sts = []
            for i in range(NCH):
                st = stage.tile([P, 512], f32, tag=f"st{i}")
                nc.vector.tensor_add(st[:], ps0[i][:], yts[i][0][:])
                nc.vector.tensor_add(st[:], st[:], yts[i][1][:])
                sts.append(st)
            return sts

        def emit_e1(mo, sts):
            ps1 = [l2ps.tile([P, 512], f32, tag=f"l2b{i}", name=f"l2b{i}")
                   for i in range(NCH)]
            for k in range(ds_po):
                for i in range(NCH):
                    nc.tensor.matmul(
                        ps1[i][:], w2_sb[:, 1, k, ts(mo, P)],
                        h_sb[:, 1, k, ts(i, 512)],
                        start=(k == 0), stop=(k == ds_po - 1),
                    )
            for i in range(NCH):
                nc.vector.tensor_add(sts[i][:], sts[i][:], ps1[i][:])
                if has_b2:
                    nc.vector.tensor_scalar_add(
                        sts[i][:], sts[i][:], b2sum[:, mo:mo + 1])
                nc.sync.dma_start(outT_t[:, mo, ts(i, 512)], sts[i][:])

        pend = []
        for mo in range(x_po):
            pend.append((mo, emit_e0(mo)))
            if len(pend) > LAG:
                m0, sts = pend.pop(0)
                emit_e1(m0, sts)
        for m0, sts in pend:
            emit_e1(m0, sts)
    nc.compile()
    return nc


# --------------------------------------------------------------------------
# Phase 3: combine out = shared + y0 + y1
# --------------------------------------------------------------------------

def build_p3(cfg: Cfg, debug: bool = False, loop_n: int = 0):
    nc = bacc.Bacc("TRN2", target_bir_lowering=False, debug=debug)
    f32 = dt.float32
    aT = nc.dram_tensor("aT", [cfg.D, cfg.TPC], f32, kind="ExternalInput").ap()
    bT = nc.dram_tensor("bT", [cfg.D, cfg.TPC], dt.bfloat16, kind="ExternalInput").ap()
    cT = nc.dram_tensor("cT", [cfg.D, cfg.TPC], dt.bfloat16, kind="ExternalInput").ap()
    oT = nc.dram_tensor("oT", [cfg.D, cfg.TPC], f32, kind="ExternalOutput").ap()

    x_po = cfg.D // P
    CH = 128
    with tile.TileContext(nc) as tc, _maybe_loop(tc, loop_n), ExitStack() as ctx:
        pool = ctx.enter_context(tc.tile_pool(name="sb", bufs=3))
        aT_t, bT_t, cT_t, oT_t = _rearr2(aT), _rearr2(bT), _rearr2(cT), _rearr2(oT)
        for c in range(cfg.TPC // CH):
            a = pool.tile([P, x_po, CH], f32, tag="a")
            nc.sync.dma_start(a[:], aT_t[:, :, ts(c, CH)])
            b = pool.tile([P, x_po, CH], f32, tag="b")
            nc.gpsimd.dma_start(b[:], bT_t[:, :, ts(c, CH)])  # bf16 -> f32 cast
            cc = pool.tile([P, x_po, CH], f32, tag="c")
            nc.gpsimd.dma_start(cc[:], cT_t[:, :, ts(c, CH)])
            nc.vector.tensor_add(a[:], a[:], b[:])
            nc.vector.tensor_add(a[:], a[:], cc[:])
            nc.sync.dma_start(oT_t[:, :, ts(c, CH)], a[:])
    nc.compile()
    return nc


# --------------------------------------------------------------------------
# Host orchestration
# --------------------------------------------------------------------------

def _get(phase: str, cfg: Cfg, **bkw):
    key = (phase, cfg, tuple(sorted(bkw.items())))
    if key not in _cache:
        _cache[key] = {
            "p1": build_p1, "p2": build_p2, "p3": build_p3,
            "p1r": build_p1r, "p3s": build_p3s, "pr": build_pr,
            "p2w": build_p2w,
            "p1f": build_p1f, "p2f": build_p2f, "p3f": build_p3f,
            "p1g": build_p1g,
        }[phase](cfg, **bkw)
    return _cache[key]


def _run(phase: str, cfg: Cfg, in_maps, **bkw):
    nc = _get(phase, cfg, **bkw)
    r = run_bass_kernel_spmd(nc, in_maps, core_ids=list(range(cfg.n_cores)), trace=TRACE)
    LAST_EXEC_NS[phase] = r.exec_time_ns
    return r.results


def kernel(**inputs) -> np.ndarray:
    cfg = CFG
    if np.any(np.asarray(inputs["re_b2"])):
        return _kernel_fallback(**inputs)
    E4NP = dt.np(dt.float8e4)
    x = np.ascontiguousarray(np.asarray(inputs["x"], dtype=np.float32))
    Bn, S, D = x.shape
    assert (Bn, S, D) == (cfg.n_cores, cfg.TPC, cfg.D)
    step_t = int(np.asarray(inputs["step_t"]))
    rw = np.ascontiguousarray(np.asarray(inputs["router_w"], np.float32)[step_t])
    re_b1 = np.ascontiguousarray(np.asarray(inputs["re_b1"], np.float32))
    w1f8 = np.ascontiguousarray(
        (np.asarray(inputs["re_w1"], np.float32) * 16.0).astype(E4NP))
    w2f8 = np.ascontiguousarray(
        (np.asarray(inputs["re_w2"], np.float32) * 16.0).astype(E4NP))
    sh_w1 = np.ascontiguousarray(np.asarray(inputs["sh_w1"], np.float32).astype(BF16))
    sh_b1 = np.ascontiguousarray(np.asarray(inputs["sh_b1"], np.float32))
    sh_w2 = np.ascontiguousarray(np.asarray(inputs["sh_w2"], np.float32).astype(BF16))
    sh_b2 = np.ascontiguousarray(np.asarray(inputs["sh_b2"], np.float32))

    xT = np.ascontiguousarray(x.transpose(0, 2, 1))  # [B, D, S] feature-major
    xbf = xT.astype(BF16)                            # hi half (bf16)
    xlo = (xT - xbf.astype(np.float32)).astype(BF16)  # lo half (bf16)
    rwh = rw.astype(BF16)
    rwl = (rw - rwh.astype(np.float32)).astype(BF16)

    # ---- phase 1: router logits + shared layer 1 ----
    in1 = [{"xbfT": np.ascontiguousarray(xbf[b]),
            "xloT": np.ascontiguousarray(xlo[b]),
            "rwh": rwh, "rwl": rwl, "sw1": sh_w1, "sb1": sh_b1}
           for b in range(cfg.n_cores)]
    r1 = _run("p1g", cfg, in1)

    # host top-2 + softmax on the exact device logits
    logits = np.stack([r["lgT"].T for r in r1])       # [B, S, E] f32
    idx = np.argsort(-logits, axis=-1, kind="stable")[..., :2].astype(np.int64)
    mx = logits.max(-1, keepdims=True)
    el = np.exp(logits - mx)
    probs = el / el.sum(-1, keepdims=True)
    wts = np.take_along_axis(probs, idx, axis=-1).astype(np.float32)  # [B, S, 2]

    T = Bn * S
    pair_e = idx.reshape(-1)                   # expert of pair p (p = g*2 + k)
    order = np.argsort(pair_e, kind="stable")  # pairs sorted by expert
    counts = np.bincount(pair_e, minlength=cfg.E)

    # slot assignment: 8 most-loaded experts -> slot A, rest -> slot B
    eorder = np.argsort(-counts, kind="stable")
    slotA, slotB = eorder[:cfg.n_cores], eorder[cfg.n_cores:]
    pad512 = lambda n: max(512, int(-(-int(n) // 512) * 512))
    if counts[slotA].max() > cfg.CAP_A or counts[slotB].max() > cfg.CAP_B:
        cfg = Cfg(CAP_A=pad512(counts[slotA].max()),
                  CAP_B=pad512(counts[slotB].max()))
    W = cfg.W2
    core_of = np.empty(cfg.E, np.int64)
    slot_off = np.empty(cfg.E, np.int64)
    core_of[slotA] = np.arange(cfg.n_cores); slot_off[slotA] = 0
    core_of[slotB] = np.arange(cfg.n_cores); slot_off[slotB] = cfg.CAP_A

    xball = np.concatenate(
        [xT[b].astype(BF16) for b in range(Bn)], axis=1).astype(E4NP)  # [D, T]

    seg = np.zeros(cfg.E + 1, np.int64)
    seg[1:] = np.cumsum(counts)
    caps = np.where(slot_off == 0, cfg.CAP_A, cfg.CAP_B)
    cols = [np.zeros(int(caps[e]), np.int64) for e in range(cfg.E)]
    cwv = [np.zeros(int(caps[e]), np.float32) for e in range(cfg.E)]
    pos_of_pair = np.empty(2 * T, np.int64)
    wflat = wts.reshape(-1)
    for e in range(cfg.E):
        sl = order[seg[e]:seg[e + 1]]
        n = len(sl)
        cols[e][:n] = sl // 2
        cwv[e][:n] = wflat[sl] / 16.0          # fold w2's 16x fp8 scale
        pos_of_pair[sl] = np.arange(n)

    # ---- phase 2: routed experts (fp8) ----
    in2 = []
    for c in range(cfg.n_cores):
        eA, eB = int(slotA[c]), int(slotB[c])
        xg = xball[:, np.concatenate([cols[eA], cols[eB]])]
        in2.append({
            "xgT": np.ascontiguousarray(xg),
            "ew1": w1f8[[eA, eB]],
            "eb1": re_b1[[eA, eB]],
            "ew2": w2f8[[eA, eB]],
            "cw": np.concatenate([cwv[eA], cwv[eB]]),
        })
    r2 = _run("p2f", cfg, in2)

    yall = np.concatenate([r["ygT"] for r in r2], axis=1)  # [D, B*W] bf16
    ycol_of_pair = core_of[pair_e] * W + slot_off[pair_e] + pos_of_pair
    ya = yall[:, ycol_of_pair[0::2]]                       # [D, T] slot k=0
    yb = yall[:, ycol_of_pair[1::2]]                       # [D, T] slot k=1

    # ---- phase 3: shared layer 2 + combine ----
    in3 = [
        {
            "hT": r1[b]["hT"],
            "sw2": sh_w2, "sb2": sh_b2,
            "yaT": np.ascontiguousarray(ya[:, b * S:(b + 1) * S]),
            "ybT": np.ascontiguousarray(yb[:, b * S:(b + 1) * S]),
        }
        for b in range(cfg.n_cores)
    ]
    r3 = _run("p3f", cfg, in3, has_b2=bool(np.any(sh_b2)))

    out = np.stack([r["outT"] for r in r3])                # [B, D, S] f32
    return np.ascontiguousarray(out.transpose(0, 2, 1))   # [B, S, D] f32


def _kernel_fallback(**inputs) -> np.ndarray:
    cfg = CFG
    x = np.ascontiguousarray(np.asarray(inputs["x"], dtype=np.float32))
    Bn, S, D = x.shape
    assert (Bn, S, D) == (cfg.n_cores, cfg.TPC, cfg.D)
    step_t = int(np.asarray(inputs["step_t"]))
    rw = np.ascontiguousarray(np.asarray(inputs["router_w"], np.float32)[step_t])
    re_w1 = np.ascontiguousarray(np.asarray(inputs["re_w1"], np.float32).astype(BF16))
    re_b1 = np.ascontiguousarray(np.asarray(inputs["re_b1"], np.float32))
    re_w2 = np.ascontiguousarray(np.asarray(inputs["re_w2"], np.float32).astype(BF16))
    re_b2 = np.ascontiguousarray(np.asarray(inputs["re_b2"], np.float32))
    sh_w1 = np.ascontiguousarray(np.asarray(inputs["sh_w1"], np.float32).astype(BF16))
    sh_b1 = np.ascontiguousarray(np.asarray(inputs["sh_b1"], np.float32))
    sh_w2 = np.ascontiguousarray(np.asarray(inputs["sh_w2"], np.float32).astype(BF16))
    sh_b2 = np.ascontiguousarray(np.asarray(inputs["sh_b2"], np.float32))

    xT = np.ascontiguousarray(x.transpose(0, 2, 1))  # [B, D, S] feature-major
    xbfT = xT.astype(BF16)                           # device compute dtype

    # ---- phase 1: router ----
    in1 = [{"xT": xT[b], "rw": rw} for b in range(cfg.n_cores)]
    r1 = _run("pr", cfg, in1)

    idx = np.stack([r["ridx"][:, :2] for r in r1]).astype(np.int64)   # [B, S, 2]
    wts = np.stack([r["rwts"][:, :2] for r in r1])                    # [B, S, 2] f32
    xball = np.concatenate(list(xbfT), axis=1)                        # [D, T] bf16

    T = Bn * S
    pair_e = idx.reshape(-1)                   # expert of pair p (p = g*2 + k)
    order = np.argsort(pair_e, kind="stable")  # pairs sorted by expert
    counts = np.bincount(pair_e, minlength=cfg.E)

    if counts.max() > cfg.CAP:  # safety net: regrow capacity, rebuild p2
        cfg = Cfg(CAP=int(-(-(counts.max() + 64) // P) * P))

    seg = np.zeros(cfg.E + 1, np.int64)
    seg[1:] = np.cumsum(counts)
    cols = np.zeros((cfg.E, cfg.CAP), np.int64)           # token col in xball
    cwarr = np.zeros((cfg.E, cfg.CAPP), np.float32)       # combine weights
    pos_of_pair = np.empty(2 * T, np.int64)
    wflat = wts.reshape(-1)
    for e in range(cfg.E):
        sl = order[seg[e]:seg[e + 1]]
        n = len(sl)
        cols[e, :n] = sl // 2
        cwarr[e, :n] = wflat[sl]
        pos_of_pair[sl] = np.arange(n)

    # ---- phase 2 ----
    in2 = []
    for c in range(cfg.n_cores):
        e0 = c * cfg.EPC
        xg = xball[:, cols[e0:e0 + cfg.EPC].reshape(-1)]  # [D, EPC*CAP] bf16
        in2.append({
            "xgT": np.ascontiguousarray(xg),
            "ew1": re_w1[e0:e0 + cfg.EPC],
            "eb1": re_b1[e0:e0 + cfg.EPC],
            "ew2": re_w2[e0:e0 + cfg.EPC],
            "eb2": re_b2[e0:e0 + cfg.EPC],
            "cw": cwarr[e0:e0 + cfg.EPC],
        })
    has_b2 = bool(np.any(np.asarray(inputs["re_b2"])))
    if has_b2:
        r2 = _run("p2", cfg, in2, has_b2=True)
    else:
        for m in in2:
            m.pop("eb2")
        r2 = _run("p2w", cfg, in2)

    # global y layout: expert e occupies columns [e*CAP, (e+1)*CAP)
    yall = np.concatenate([r["ygT"] for r in r2], axis=1)  # [D, E*CAP] bf16

    ycol_of_pair = pair_e * cfg.CAP + pos_of_pair          # [2T]
    ya = yall[:, ycol_of_pair[0::2]]                       # [D, T] slot k=0
    yb = yall[:, ycol_of_pair[1::2]]                       # [D, T] slot k=1

    # ---- phase 3: shared experts + combine ----
    in3 = [
        {
            "xbfT": xbfT[b],
            "sw1": sh_w1, "sb1": sh_b1, "sw2": sh_w2, "sb2": sh_b2,
            "yaT": np.ascontiguousarray(ya[:, b * S:(b + 1) * S]),
            "ybT": np.ascontiguousarray(yb[:, b * S:(b + 1) * S]),
        }
        for b in range(cfg.n_cores)
    ]
    r3 = _run("p3s", cfg, in3, has_b2=bool(np.any(sh_b2)))

    out = np.stack([r["outT"] for r in r3]).astype(np.float32)  # [B, D, S]
    return np.ascontiguousarray(out.transpose(0, 2, 1))    # [B, S, D] f32



# revision 22
# speedup vs baseline: 1.2643x; 1.2643x over previous
"""Trainium2 Bass kernel for nn_ChainOfExperts (MoE with shared experts).

Strategy (8 NeuronCores):
  Phase pr (data-parallel, tokens sharded along B): router logits (f32 for
    exact top-2 agreement with the reference) + top-2 softmax weights.
  Host: pure data movement/layout — bf16 cast of x, bf16 cast of weights,
    group token slots by routed expert (counting sort on device-computed
    indices), gather bf16 token columns per expert.
  Phase p2w (expert-parallel, 2 experts per core): weight-stationary
    routed-expert FFN on the gathered tokens (capacity-padded); both
    experts' w1/w2 live in SBUF so the PE never stalls on weight DMA;
    output pre-scaled by combine weight.
  Phase p3s (data-parallel): shared-expert FFN (single M-batched layer-1
    composable over both shared experts, h kept in SBUF) fused with the
    final combine out = shared + y_slot0 + y_slot1.

All activations are kept feature-major ([D, tokens]) so every matmul has its
contraction dim on partitions. Matmuls run in bf16 (fp32 accumulate); the
router runs in fp32.
"""

import os
from contextlib import ExitStack
from dataclasses import dataclass

import numpy as np
import ml_dtypes

import concourse.bass as bass
import concourse.mybir as mybir
import concourse.tile as tile
from concourse import bacc
from concourse.bass import ts, ds
from concourse.bass_utils import run_bass_kernel_spmd
from concourse.kernels.tile_matmul import (
    ShapeInfo,
    composable_matmul_tile_kernel,
    cast_to_type,
    dma_from_dram_kxm,
    dma_from_dram_kxn,
    dma_to_dram_mxn,
)

BF16 = ml_dtypes.bfloat16
ActFn = mybir.ActivationFunctionType
dt = mybir.dt
P = 128

# bass_utils imports antenv.axon_hooks when tracing is requested; this
# container ships only an antenv stub. Provide the missing module so a
# trace request degrades to an untraced run instead of crashing.
import sys as _sys
try:
    import antenv.axon_hooks  # noqa: F401
except ImportError:
    import types as _types
    import antenv as _antenv
    _stub = _types.ModuleType("antenv.axon_hooks")
    _stub.get_axon_ntff_profile_hook = lambda: None
    _sys.modules["antenv.axon_hooks"] = _stub
    _antenv.axon_hooks = _stub


@dataclass(frozen=True)
class Cfg:
    n_cores: int = 8
    D: int = 2048     # hidden dim
    TPC: int = 2048   # tokens per core
    E: int = 16       # routed experts
    NSH: int = 2      # shared experts
    DS: int = 1024    # shared inner dim
    DR: int = 512     # routed inner dim
    CAP: int = 2304   # per-expert token capacity (multiple of 128)
    EPC: int = 2      # experts per core
    CAP_A: int = 2304  # slot-A capacity (8 most-loaded experts)
    CAP_B: int = 2048  # slot-B capacity (8 least-loaded experts)

    @property
    def n_tile(self):  # composable's N tiling for N=CAP
        return min(512, -(-self.CAP // P) * P)

    @property
    def n_tiles(self):
        return -(-self.CAP // self.n_tile)

    @property
    def CAPP(self):  # hg cache width: CAP padded to whole n-tiles
        return self.n_tiles * self.n_tile

    @property
    def W2(self):  # p2f token-slot width
        return self.CAP_A + self.CAP_B


CFG = Cfg()
TRACE = bool(os.environ.get("KERNEL_TRACE"))
LAST_EXEC_NS: dict[str, int | None] = {}

_cache: dict = {}


def _rearr2(ap):
    """[K, N] dram AP -> [pi, po, N] with K = po*128 + pi."""
    return ap.rearrange("(po pi) t -> pi po t", pi=P)


# --------------------------------------------------------------------------
# Phase 1: router + shared experts + bf16 cast of x
# --------------------------------------------------------------------------

class _NullCtx:
    def __enter__(self):
        return None

    def __exit__(self, *a):
        return False


def _maybe_loop(tc, loop_n):
    """Wrap the phase body in an in-NEFF repeat loop (for benchmarking)."""
    return tc.For_i(0, loop_n, 1) if loop_n else _NullCtx()


def build_p1(cfg: Cfg, debug: bool = False, loop_n: int = 0):
    nc = bacc.Bacc("TRN2", target_bir_lowering=False, debug=debug)
    f32 = dt.float32
    xT = nc.dram_tensor("xT", [cfg.D, cfg.TPC], f32, kind="ExternalInput").ap()
    rw = nc.dram_tensor("rw", [cfg.D, cfg.E], f32, kind="ExternalInput").ap()
    sw1 = nc.dram_tensor("sw1", [cfg.NSH, cfg.D, cfg.DS], f32, kind="ExternalInput").ap()
    sb1 = nc.dram_tensor("sb1", [cfg.NSH, cfg.DS], f32, kind="ExternalInput").ap()
    sw2 = nc.dram_tensor("sw2", [cfg.NSH, cfg.DS, cfg.D], f32, kind="ExternalInput").ap()
    sb2 = nc.dram_tensor("sb2", [cfg.NSH, cfg.D], f32, kind="ExternalInput").ap()
    out_shT = nc.dram_tensor("out_shT", [cfg.D, cfg.TPC], f32, kind="ExternalOutput").ap()
    xbfT = nc.dram_tensor("xbfT", [cfg.D, cfg.TPC], dt.bfloat16, kind="ExternalOutput").ap()
    ridx = nc.dram_tensor("ridx", [cfg.TPC, 8], dt.uint32, kind="ExternalOutput").ap()
    rwts = nc.dram_tensor("rwts", [cfg.TPC, 8], f32, kind="ExternalOutput").ap()
    h_dram = nc.dram_tensor("h_mid", [cfg.NSH, cfg.DS, cfg.TPC], dt.bfloat16).ap()

    x_po = cfg.D // P
    ds_po = cfg.DS // P
    CH = 256  # router/cast chunk (tokens)

    with tile.TileContext(nc) as tc, _maybe_loop(tc, loop_n), ExitStack() as ctx:
        const = ctx.enter_context(tc.tile_pool(name="const", bufs=1))
        rw_sb = const.tile([P, x_po, cfg.E], f32)
        nc.sync.dma_start(rw_sb[:], rw.rearrange("(po pi) e -> pi po e", pi=P))
        b1_sb = const.tile([P, cfg.NSH, ds_po], f32)
        nc.sync.dma_start(b1_sb[:], sb1.rearrange("s (po pi) -> pi s po", pi=P))
        b2_sb = const.tile([P, cfg.NSH, x_po], f32)
        nc.sync.dma_start(b2_sb[:], sb2.rearrange("s (po pi) -> pi s po", pi=P))
        b2sum = const.tile([P, x_po], f32)
        nc.vector.tensor_add(b2sum[:], b2_sb[:, 0], b2_sb[:, 1])
        xbf_cache = const.tile([P, x_po, cfg.TPC], dt.bfloat16)

        # ---- router + cast pass ----
        with ExitStack() as c2:
            xsrc = c2.enter_context(tc.tile_pool(name="xsrc", bufs=2))
            rps = c2.enter_context(tc.tile_pool(name="rpsum", bufs=2, space="PSUM"))
            rsb = c2.enter_context(tc.tile_pool(name="rsb", bufs=3))
            xT_t = _rearr2(xT)
            xbfT_t = _rearr2(xbfT)
            for c in range(cfg.TPC // CH):
                xt = xsrc.tile([P, x_po, CH], f32, tag="xt")
                nc.sync.dma_start(xt[:], xT_t[:, :, ts(c, CH)])
                nc.vector.tensor_copy(xbf_cache[:, :, ts(c, CH)], xt[:])
                nc.sync.dma_start(xbfT_t[:, :, ts(c, CH)], xbf_cache[:, :, ts(c, CH)])
                for tt in range(CH // P):
                    t0 = c * CH + tt * P
                    ps = rps.tile([P, cfg.E], f32, tag="rp")
                    for po in range(x_po):
                        nc.tensor.matmul(
                            ps[:], xt[:, po, ts(tt, P)], rw_sb[:, po, :],
                            start=(po == 0), stop=(po == x_po - 1),
                        )
                    lg = rsb.tile([P, cfg.E], f32, tag="lg")
                    nc.vector.tensor_copy(lg[:], ps[:])
                    mx = rsb.tile([P, 8], f32, tag="mx")
                    nc.vector.max(mx[:], lg[:])
                    ix = rsb.tile([P, 8], dt.uint32, tag="ix")
                    nc.vector.max_index(ix[:], mx[:], lg[:])
                    nm = rsb.tile([P, 1], f32, tag="nm")
                    nc.vector.tensor_scalar_mul(nm[:], mx[:, 0:1], -1.0)
                    ex = rsb.tile([P, cfg.E], f32, tag="ex")
                    zz = rsb.tile([P, 1], f32, tag="zz")
                    nc.scalar.activation(ex[:], lg[:], ActFn.Exp, bias=nm[:], accum_out=zz[:])
                    rz = rsb.tile([P, 1], f32, tag="rz")
                    nc.vector.reciprocal(rz[:], zz[:])
                    wv = rsb.tile([P, 8], f32, tag="wv")
                    nc.scalar.activation(wv[:], mx[:], ActFn.Exp, bias=nm[:])
                    nc.vector.tensor_scalar_mul(wv[:], wv[:], rz[:])
                    nc.sync.dma_start(ridx[ds(t0, P), :], ix[:])
                    nc.sync.dma_start(rwts[ds(t0, P), :], wv[:])

        # ---- shared experts layer 1 (per shared expert s) ----
        xbf_shape = ShapeInfo(pdims=((P, x_po),), fdims=(cfg.TPC,))

        def xbf_producer(nc_, md):
            return xbf_cache[
                :, ts(md.k_tile_idx, md.k_subtiles),
                ds(md.n_tile_idx * md.n_tile, md.n_tile)
            ]

        for s in range(cfg.NSH):
            with ExitStack() as c2:
                mpool = c2.enter_context(tc.tile_pool(name=f"l1m{s}", bufs=2))
                cpool = c2.enter_context(tc.tile_pool(name=f"l1c{s}", bufs=5))
                kxm_prod, kxm_shape = dma_from_dram_kxm(mpool, sw1[s])
                kxm_prod = cast_to_type(kxm_prod, cpool, dt.bfloat16)

                def l1_reducer(nc_, psum, sbuf, md, s=s):
                    ko = (md.m_tile_idx * md.m_tile + md.m_subtile_idx * P) // P
                    nc_.scalar.activation(
                        sbuf[:], psum[:], ActFn.Silu, bias=b1_sb[:, s, ko:ko + 1]
                    )

                composable_matmul_tile_kernel(
                    tc=tc,
                    kxm_shape=kxm_shape,
                    kxn_shape=xbf_shape,
                    output_type=dt.bfloat16,
                    kxm_producer=kxm_prod,
                    kxn_producer=xbf_producer,
                    mxn_consumer=dma_to_dram_mxn(h_dram[s]),
                    mxn_subtile_reducer=l1_reducer,
                )

        # ---- shared experts layer 2 (contract over s and DS jointly) ----
        with ExitStack() as c2:
            mpool = c2.enter_context(tc.tile_pool(name="l2m", bufs=2))
            cpool = c2.enter_context(tc.tile_pool(name="l2c", bufs=5))
            npool = c2.enter_context(tc.tile_pool(name="l2n", bufs=5))
            kxm_prod, kxm_shape = dma_from_dram_kxm(mpool, sw2, batch_k=True)
            kxm_prod = cast_to_type(kxm_prod, cpool, dt.bfloat16)
            kxn_prod, kxn_shape = dma_from_dram_kxn(npool, h_dram, batch_k=True)

            def l2_reducer(nc_, psum, sbuf, md):
                do = (md.m_tile_idx * md.m_tile + md.m_subtile_idx * P) // P
                nc_.vector.tensor_scalar_add(sbuf[:], psum[:], b2sum[:, do:do + 1])

            composable_matmul_tile_kernel(
                tc=tc,
                kxm_shape=kxm_shape,
                kxn_shape=kxn_shape,
                output_type=dt.float32,
                kxm_producer=kxm_prod,
                kxn_producer=kxn_prod,
                mxn_consumer=dma_to_dram_mxn(out_shT),
                mxn_subtile_reducer=l2_reducer,
            )

    nc.compile()
    return nc


def build_p1r(cfg: Cfg, debug: bool = False, loop_n: int = 0):
    """Router-only phase: top-2 indices/weights + bf16 cast of x.

    x is loaded po-sliced with full token rows (8KB contiguous runs) for DMA
    efficiency. Each 128-token group's logits use a private PSUM tile per
    po-slice (complete start/stop groups) and accumulate in SBUF, avoiding
    interleaved-group and PSUM bank-sharing hazards.
    """
    nc = bacc.Bacc("TRN2", target_bir_lowering=False, debug=debug)
    f32 = dt.float32
    xT = nc.dram_tensor("xT", [cfg.D, cfg.TPC], f32, kind="ExternalInput").ap()
    rw = nc.dram_tensor("rw", [cfg.D, cfg.E], f32, kind="ExternalInput").ap()
    xbfT = nc.dram_tensor("xbfT", [cfg.D, cfg.TPC], dt.bfloat16, kind="ExternalOutput").ap()
    ridx = nc.dram_tensor("ridx", [cfg.TPC, 8], dt.uint32, kind="ExternalOutput").ap()
    rwts = nc.dram_tensor("rwts", [cfg.TPC, 8], f32, kind="ExternalOutput").ap()

    x_po = cfg.D // P
    PO_CH = 2
    NPO = x_po // PO_CH
    NG = cfg.TPC // P

    with tile.TileContext(nc) as tc, _maybe_loop(tc, loop_n), ExitStack() as ctx:
        const = ctx.enter_context(tc.tile_pool(name="const", bufs=1))
        rw_sb = const.tile([P, x_po, cfg.E], f32)
        nc.sync.dma_start(rw_sb[:], rw.rearrange("(po pi) e -> pi po e", pi=P))
        lg_acc = const.tile([P, NG, cfg.E], f32)
        xsrc = ctx.enter_context(tc.tile_pool(name="xsrc", bufs=3))
        xbfp = ctx.enter_context(tc.tile_pool(name="xbfp", bufs=3))
        rps = ctx.enter_context(tc.tile_pool(name="rpsum", bufs=4, space="PSUM"))
        rsb = ctx.enter_context(tc.tile_pool(name="rsb", bufs=3))
        xT_t = _rearr2(xT)
        xbfT_t = _rearr2(xbfT)

        for po8 in range(NPO):
            xt = xsrc.tile([P, PO_CH, cfg.TPC], f32, tag="xt")
            nc.sync.dma_start(xt[:], xT_t[:, ts(po8, PO_CH), :])
            xb = xbfp.tile([P, PO_CH, cfg.TPC], dt.bfloat16, tag="xb")
            nc.vector.tensor_copy(xb[:], xt[:])
            nc.sync.dma_start(xbfT_t[:, ts(po8, PO_CH), :], xb[:])
            for tt in range(NG):
                ps = rps.tile([P, cfg.E], f32, tag="rp")
                for pp in range(PO_CH):
                    nc.tensor.matmul(
                        ps[:], xt[:, pp, ts(tt, P)],
                        rw_sb[:, po8 * PO_CH + pp, :],
                        start=(pp == 0), stop=(pp == PO_CH - 1),
                    )
                if po8 == 0:
                    nc.vector.tensor_copy(lg_acc[:, tt, :], ps[:])
                else:
                    nc.vector.tensor_add(lg_acc[:, tt, :], lg_acc[:, tt, :], ps[:])

        for tt in range(NG):
            t0 = tt * P
            lg = lg_acc[:, tt, :]
            mx = rsb.tile([P, 8], f32, tag="mx")
            nc.vector.max(mx[:], lg)
            ix = rsb.tile([P, 8], dt.uint32, tag="ix")
            nc.vector.max_index(ix[:], mx[:], lg)
            nm = rsb.tile([P, 1], f32, tag="nm")
            nc.vector.tensor_scalar_mul(nm[:], mx[:, 0:1], -1.0)
            ex = rsb.tile([P, cfg.E], f32, tag="ex")
            zz = rsb.tile([P, 1], f32, tag="zz")
            nc.scalar.activation(ex[:], lg, ActFn.Exp, bias=nm[:], accum_out=zz[:])
            rz = rsb.tile([P, 1], f32, tag="rz")
            nc.vector.reciprocal(rz[:], zz[:])
            wv = rsb.tile([P, 8], f32, tag="wv")
            nc.scalar.activation(wv[:], mx[:], ActFn.Exp, bias=nm[:])
            nc.vector.tensor_scalar_mul(wv[:], wv[:], rz[:])
            nc.sync.dma_start(ridx[ds(t0, P), :], ix[:])
            nc.sync.dma_start(rwts[ds(t0, P), :], wv[:])
    nc.compile()
    return nc


def _w_producer_batched(pool, w_ap, tagname):
    """Batched-K variant of _w_producer for [S, K, M] weights.

    f32 weights stream through the SWDGE cast-DMA; bf16 weights take the
    plain HWDGE path."""
    S, K, M = w_ap.shape
    shape = ShapeInfo(pdims=((P, K // P),) * S, fdims=(M,))
    w_ts = [w_ap[s].rearrange("(po pi) m -> pi po m", pi=P) for s in range(S)]
    is_bf16 = w_ap.dtype == dt.bfloat16

    def prod(nc_, md):
        t = pool.tile([P, md.k_subtiles, md.m_tile], dt.bfloat16, tag=tagname)
        eng = nc_.sync if is_bf16 else nc_.gpsimd
        eng.dma_start(
            t[:],
            w_ts[md.k_batch_idx][
                :, ts(md.k_tile_idx, md.k_subtiles),
                ds(md.m_tile_idx * md.m_tile, md.m_tile)
            ],
        )
        return t

    return prod, shape


def _w_producer_mbatched(pool, w_ap, tagname):
    """M-batched producer over [S, K, M] weights: fdims=(M,)*S, so one
    composable call covers all S experts' layer-1 matmuls (shared kxn)."""
    S, K, M = w_ap.shape
    shape = ShapeInfo(pdims=((P, K // P),), fdims=(M,) * S)
    w_ts = [w_ap[s].rearrange("(po pi) m -> pi po m", pi=P) for s in range(S)]
    is_bf16 = w_ap.dtype == dt.bfloat16

    def prod(nc_, md):
        t = pool.tile([P, md.k_subtiles, md.m_tile], dt.bfloat16, tag=tagname)
        eng = nc_.sync if is_bf16 else nc_.gpsimd
        eng.dma_start(
            t[:],
            w_ts[md.m_batch_idx][
                :, ts(md.k_tile_idx, md.k_subtiles),
                ds(md.m_tile_idx * md.m_tile, md.m_tile)
            ],
        )
        return t

    return prod, shape


def build_pr(cfg: Cfg, debug: bool = False, loop_n: int = 0):
    """Router-only phase: top-2 indices/weights from f32 x. No x cast (the
    host casts x to bf16 for the gather and the shared-expert phase)."""
    nc = bacc.Bacc("TRN2", target_bir_lowering=False, debug=debug)
    f32 = dt.float32
    xT = nc.dram_tensor("xT", [cfg.D, cfg.TPC], f32, kind="ExternalInput").ap()
    rw = nc.dram_tensor("rw", [cfg.D, cfg.E], f32, kind="ExternalInput").ap()
    ridx = nc.dram_tensor("ridx", [cfg.TPC, 8], dt.uint32, kind="ExternalOutput").ap()
    rwts = nc.dram_tensor("rwts", [cfg.TPC, 8], f32, kind="ExternalOutput").ap()

    x_po = cfg.D // P
    PO_CH = 4
    NPO = x_po // PO_CH
    NG = cfg.TPC // P

    with tile.TileContext(nc) as tc, _maybe_loop(tc, loop_n), ExitStack() as ctx:
        const = ctx.enter_context(tc.tile_pool(name="const", bufs=1))
        rw_sb = const.tile([P, x_po, cfg.E], f32)
        nc.sync.dma_start(rw_sb[:], rw.rearrange("(po pi) e -> pi po e", pi=P))
        lg_acc = const.tile([P, NG, cfg.E], f32)
        xsrc = ctx.enter_context(tc.tile_pool(name="xsrc", bufs=2))
        rps = ctx.enter_context(tc.tile_pool(name="rpsum", bufs=4, space="PSUM"))
        rsb = ctx.enter_context(tc.tile_pool(name="rsb", bufs=3))
        xT_t = _rearr2(xT)

        for po8 in range(NPO):
            xt = xsrc.tile([P, PO_CH, cfg.TPC], f32, tag="xt")
            nc.sync.dma_start(xt[:], xT_t[:, ts(po8, PO_CH), :])
            for tt in range(NG):
                ps = rps.tile([P, cfg.E], f32, tag="rp")
                for pp in range(PO_CH):
                    nc.tensor.matmul(
                        ps[:], xt[:, pp, ts(tt, P)],
                        rw_sb[:, po8 * PO_CH + pp, :],
                        start=(pp == 0), stop=(pp == PO_CH - 1),
                    )
                if po8 == 0:
                    nc.vector.tensor_copy(lg_acc[:, tt, :], ps[:])
                else:
                    nc.vector.tensor_add(lg_acc[:, tt, :], lg_acc[:, tt, :], ps[:])

        for tt in range(NG):
            t0 = tt * P
            lg = lg_acc[:, tt, :]
            mx = rsb.tile([P, 8], f32, tag="mx")
            nc.vector.max(mx[:], lg)
            ix = rsb.tile([P, 8], dt.uint32, tag="ix")
            nc.vector.max_index(ix[:], mx[:], lg)
            nm = rsb.tile([P, 1], f32, tag="nm")
            nc.vector.tensor_scalar_mul(nm[:], mx[:, 0:1], -1.0)
            ex = rsb.tile([P, cfg.E], f32, tag="ex")
            zz = rsb.tile([P, 1], f32, tag="zz")
            nc.scalar.activation(ex[:], lg, ActFn.Exp, bias=nm[:], accum_out=zz[:])
            rz = rsb.tile([P, 1], f32, tag="rz")
            nc.vector.reciprocal(rz[:], zz[:])
            wv = rsb.tile([P, 8], f32, tag="wv")
            nc.scalar.activation(wv[:], mx[:], ActFn.Exp, bias=nm[:])
            nc.vector.tensor_scalar_mul(wv[:], wv[:], rz[:])
            nc.sync.dma_start(ridx[ds(t0, P), :], ix[:])
            nc.sync.dma_start(rwts[ds(t0, P), :], wv[:])
    nc.compile()
    return nc


def build_p3s(cfg: Cfg, debug: bool = False, loop_n: int = 0, has_b2: bool = False):
    """Shared experts + combine: out = sharedFFN(x) + ya + yb.

    h is kept in SBUF as per-(s, n-tile) tiles so layer 2 pipelines with
    layer 1 at tile granularity (no DRAM roundtrip, no coarse-dep stall).
    """
    nc = bacc.Bacc("TRN2", target_bir_lowering=False, debug=debug)
    f32 = dt.float32
    bf16 = dt.bfloat16
    xbfT = nc.dram_tensor("xbfT", [cfg.D, cfg.TPC], dt.bfloat16, kind="ExternalInput").ap()
    sw1 = nc.dram_tensor("sw1", [cfg.NSH, cfg.D, cfg.DS], bf16, kind="ExternalInput").ap()
    sb1 = nc.dram_tensor("sb1", [cfg.NSH, cfg.DS], f32, kind="ExternalInput").ap()
    sw2 = nc.dram_tensor("sw2", [cfg.NSH, cfg.DS, cfg.D], bf16, kind="ExternalInput").ap()
    sb2 = nc.dram_tensor("sb2", [cfg.NSH, cfg.D], f32, kind="ExternalInput").ap()
    yaT = nc.dram_tensor("yaT", [cfg.D, cfg.TPC], dt.bfloat16, kind="ExternalInput").ap()
    ybT = nc.dram_tensor("ybT", [cfg.D, cfg.TPC], dt.bfloat16, kind="ExternalInput").ap()
    outT = nc.dram_tensor("outT", [cfg.D, cfg.TPC], bf16, kind="ExternalOutput").ap()

    x_po = cfg.D // P
    ds_po = cfg.DS // P
    NT = cfg.TPC // 512  # n tiles

    with tile.TileContext(nc) as tc, _maybe_loop(tc, loop_n), ExitStack() as ctx:
        const = ctx.enter_context(tc.tile_pool(name="const", bufs=1))
        b1_sb = const.tile([P, cfg.NSH, ds_po], f32)
        nc.sync.dma_start(b1_sb[:], sb1.rearrange("s (po pi) -> pi s po", pi=P))
        b2_sb = const.tile([P, cfg.NSH, x_po], f32)
        nc.sync.dma_start(b2_sb[:], sb2.rearrange("s (po pi) -> pi s po", pi=P))
        b2sum = const.tile([P, x_po], f32)
        nc.vector.tensor_add(b2sum[:], b2_sb[:, 0], b2_sb[:, 1])

        hpool = ctx.enter_context(tc.tile_pool(name="hp", bufs=1))
        h_tiles = [
            [
                hpool.tile([P, ds_po, 512], dt.bfloat16,
                           tag=f"h{s}_{n}", name=f"h{s}_{n}")
                for n in range(NT)
            ]
            for s in range(cfg.NSH)
        ]

        # x cached whole in SBUF: the composable would otherwise re-DMA each
        # kxn tile once per m_outer (4x24MB of reads for this shape).
        xsb = const.tile([P, x_po, cfg.TPC], bf16)
        nc.sync.dma_start(xsb[:], _rearr2(xbfT))

        with ExitStack() as c2:
            mpool = c2.enter_context(tc.tile_pool(name="l1m", bufs=3))
            kxm_prod, kxm_shape = _w_producer_mbatched(mpool, sw1, "sw1")
            kxn_shape = ShapeInfo(pdims=((P, x_po),), fdims=(cfg.TPC,))

            def x_kxn_producer(nc_, md):
                return xsb[:, ts(md.k_tile_idx, md.k_subtiles),
                           ts(md.n_tile_idx, md.n_tile)]

            kxn_prod = x_kxn_producer

            def l1_reducer(nc_, psum, sbuf, md):
                ko = (md.m_tile_idx * md.m_tile + md.m_subtile_idx * P) // P
                nc_.scalar.activation(
                    sbuf[:], psum[:], ActFn.Silu,
                    bias=b1_sb[:, md.m_batch_idx, ko:ko + 1]
                )

            def h_producer(nc_, md):
                return h_tiles[md.m_batch_idx][md.n_tile_idx][
                    :, ds(md.m_tile_idx * (md.m_tile // P), md.m_tile // P), :
                ]

            composable_matmul_tile_kernel(
                tc=tc,
                kxm_shape=kxm_shape,
                kxn_shape=kxn_shape,
                output_type=None,
                kxm_producer=kxm_prod,
                kxn_producer=kxn_prod,
                mxn_consumer=lambda nc_, sbuf, md: None,
                mxn_subtile_reducer=l1_reducer,
                mxn_subtile_producer=h_producer,
                psum_n_bufs=2,
                MAX_K_TILE_SIZE=1024,
            )

        with ExitStack() as c2:
            mpool = c2.enter_context(tc.tile_pool(name="l2m", bufs=4))
            apool = c2.enter_context(tc.tile_pool(name="addp", bufs=3))
            kxm_prod, kxm_shape = _w_producer_batched(mpool, sw2, "sw2")
            kxn_shape = ShapeInfo(pdims=((P, ds_po),) * cfg.NSH, fdims=(cfg.TPC,))

            def h_kxn_producer(nc_, md):
                return h_tiles[md.k_batch_idx][md.n_tile_idx][
                    :, ts(md.k_tile_idx, md.k_subtiles), :
                ]

            def l2_reducer(nc_, psum, sbuf, md):
                do = md.m_tile_idx * (md.m_tile // P) + md.m_subtile_idx
                if has_b2:
                    nc_.vector.tensor_scalar_add(sbuf[:], psum[:], b2sum[:, do:do + 1])
                else:
                    nc_.vector.tensor_copy(sbuf[:], psum[:])

            base_consumer = dma_to_dram_mxn(outT)
            yaT_t, ybT_t = _rearr2(yaT), _rearr2(ybT)

            def combine_consumer(nc_, sbuf, md):
                po0 = md.m_tile_idx * (md.m_tile // P)
                nsub = md.m_tile // P
                nsl = ds(md.n_tile_idx * md.n_tile, md.n_tile)
                ya_t = apool.tile([P, nsub, md.n_tile], dt.bfloat16, tag="ya")
                nc_.sync.dma_start(ya_t[:], yaT_t[:, ds(po0, nsub), nsl])
                yb_t = apool.tile([P, nsub, md.n_tile], dt.bfloat16, tag="yb")
                nc_.sync.dma_start(yb_t[:], ybT_t[:, ds(po0, nsub), nsl])
                nc_.vector.tensor_add(sbuf[:], sbuf[:], ya_t[:])
                nc_.vector.tensor_add(sbuf[:], sbuf[:], yb_t[:])
                base_consumer(nc_, sbuf, md)

            composable_matmul_tile_kernel(
                tc=tc,
                kxm_shape=kxm_shape,
                kxn_shape=kxn_shape,
                output_type=dt.bfloat16,
                kxm_producer=kxm_prod,
                kxn_producer=h_kxn_producer,
                mxn_consumer=combine_consumer,
                mxn_subtile_reducer=l2_reducer,
                psum_n_bufs=2,
                MAX_K_TILE_SIZE=1024,
            )
    nc.compile()
    return nc


# --------------------------------------------------------------------------
# Phase 2: routed experts (expert-parallel, capacity padded)
# --------------------------------------------------------------------------

def _w_producer(pool, w_ap, tagname):
    """kxm producer streaming a [K, M] weight from DRAM as bf16. f32 source
    uses the SWDGE cast-DMA; bf16 source takes the plain HWDGE path."""
    K, M = w_ap.shape
    shape = ShapeInfo(pdims=((P, K // P),), fdims=(M,))
    w_t = w_ap.rearrange("(po pi) m -> pi po m", pi=P)
    is_bf16 = w_ap.dtype == dt.bfloat16

    def prod(nc_, md):
        t = pool.tile(
            [P, md.k_subtiles, md.m_tile], dt.bfloat16, tag=tagname
        )
        eng = nc_.sync if is_bf16 else nc_.gpsimd
        eng.dma_start(
            t[:],
            w_t[
                :, ts(md.k_tile_idx, md.k_subtiles),
                ds(md.m_tile_idx * md.m_tile, md.m_tile)
            ],
        )
        return t

    return prod, shape


def build_p2(cfg: Cfg, debug: bool = False, loop_n: int = 0, has_b2: bool = False):
    nc = bacc.Bacc("TRN2", target_bir_lowering=False, debug=debug)
    f32 = dt.float32
    W = cfg.EPC * cfg.CAP
    xgT = nc.dram_tensor("xgT", [cfg.D, W], dt.bfloat16, kind="ExternalInput").ap()
    ew1 = nc.dram_tensor("ew1", [cfg.EPC, cfg.D, cfg.DR], dt.bfloat16, kind="ExternalInput").ap()
    eb1 = nc.dram_tensor("eb1", [cfg.EPC, cfg.DR], f32, kind="ExternalInput").ap()
    ew2 = nc.dram_tensor("ew2", [cfg.EPC, cfg.DR, cfg.D], dt.bfloat16, kind="ExternalInput").ap()
    eb2 = nc.dram_tensor("eb2", [cfg.EPC, cfg.D], f32, kind="ExternalInput").ap()
    cw = nc.dram_tensor("cw", [cfg.EPC, cfg.CAPP], f32, kind="ExternalInput").ap()
    ygT = nc.dram_tensor("ygT", [cfg.D, W], dt.bfloat16, kind="ExternalOutput").ap()

    x_po = cfg.D // P
    dr_po = cfg.DR // P

    with tile.TileContext(nc) as tc, _maybe_loop(tc, loop_n), ExitStack() as ctx:
        const = ctx.enter_context(tc.tile_pool(name="const", bufs=1))
        b1_sb = const.tile([P, cfg.EPC, dr_po], f32)
        nc.sync.dma_start(b1_sb[:], eb1.rearrange("e (po pi) -> pi e po", pi=P))
        if has_b2:
            b2_sb = const.tile([P, cfg.EPC, x_po], f32)
            nc.sync.dma_start(b2_sb[:], eb2.rearrange("e (po pi) -> pi e po", pi=P))
        cwrep = const.tile([P, cfg.EPC, cfg.CAPP], f32)
        nc.sync.dma_start(
            cwrep[:],
            cw.rearrange("e c -> (e c)")[None].to_broadcast((P, cfg.EPC * cfg.CAPP)),
        )

        hg_shape = ShapeInfo(pdims=((P, dr_po),), fdims=(cfg.CAP,))
        hg_pool = ctx.enter_context(tc.tile_pool(name="hg", bufs=1))
        hg_tiles = [
            [
                hg_pool.tile([P, dr_po, cfg.n_tile], dt.bfloat16,
                             tag=f"hg{e}_{n}", name=f"hg{e}_{n}")
                for n in range(cfg.n_tiles)
            ]
            for e in range(cfg.EPC)
        ]

        # layer 1 for all experts first (keeps the PE stream dense)
        for e in range(cfg.EPC):
            with ExitStack() as c2:
                mpool = c2.enter_context(tc.tile_pool(name=f"p2m{e}", bufs=3))
                npool = c2.enter_context(tc.tile_pool(name=f"p2n{e}", bufs=5))
                kxm_prod, kxm_shape = _w_producer(mpool, ew1[e], f"w1_{e}")
                kxn_prod, kxn_shape = dma_from_dram_kxn(
                    npool, xgT[:, ds(e * cfg.CAP, cfg.CAP)]
                )

                def l1_reducer(nc_, psum, sbuf, md, e=e):
                    ko = (md.m_tile_idx * md.m_tile + md.m_subtile_idx * P) // P
                    nc_.scalar.activation(
                        sbuf[:], psum[:], ActFn.Silu, bias=b1_sb[:, e, ko:ko + 1]
                    )

                def hg_producer(nc_, md, e=e):
                    return hg_tiles[e][md.n_tile_idx][
                        :, ts(md.m_tile_idx, md.m_tile // P), :
                    ]

                composable_matmul_tile_kernel(
                    tc=tc,
                    kxm_shape=kxm_shape,
                    kxn_shape=kxn_shape,
                    output_type=None,
                    kxm_producer=kxm_prod,
                    kxn_producer=kxn_prod,
                    mxn_consumer=lambda nc_, sbuf, md: None,
                    mxn_subtile_reducer=l1_reducer,
                    mxn_subtile_producer=hg_producer,
                    psum_n_bufs=2,
                    MAX_K_TILE_SIZE=1024,
                )

        # layer 2 for all experts
        for e in range(cfg.EPC):
            with ExitStack() as c2:
                m2pool = c2.enter_context(tc.tile_pool(name=f"p2m2{e}", bufs=2))
                tpool = c2.enter_context(tc.tile_pool(name=f"p2t{e}", bufs=3))
                kxm2_prod, kxm2_shape = _w_producer(m2pool, ew2[e], f"w2_{e}")

                def hg_kxn_producer(nc_, md, e=e):
                    return hg_tiles[e][md.n_tile_idx][
                        :, ts(md.k_tile_idx, md.k_subtiles), :
                    ]

                def l2_reducer(nc_, psum, sbuf, md, e=e):
                    do = (md.m_tile_idx * md.m_tile + md.m_subtile_idx * P) // P
                    n0 = md.n_tile_idx * md.n_tile + md.n_subtile_idx * md.n_subtile
                    if has_b2:
                        stage = tpool.tile([P, md.n_subtile], dt.float32, tag="stage")
                        nc_.vector.tensor_scalar_add(
                            stage[:], psum[:], b2_sb[:, e, do:do + 1]
                        )
                        src = stage
                    else:
                        src = psum
                    nc_.vector.tensor_mul(
                        sbuf[:], src[:], cwrep[:, e, ds(n0, md.n_subtile)]
                    )

                composable_matmul_tile_kernel(
                    tc=tc,
                    kxm_shape=kxm2_shape,
                    kxn_shape=hg_shape,
                    output_type=dt.bfloat16,
                    kxm_producer=kxm2_prod,
                    kxn_producer=hg_kxn_producer,
                    mxn_consumer=dma_to_dram_mxn(ygT[:, ds(e * cfg.CAP, cfg.CAP)]),
                    mxn_subtile_reducer=l2_reducer,
                    psum_n_bufs=2,
                )

    nc.compile()
    return nc


def build_p2w(cfg: Cfg, debug: bool = False, loop_n: int = 0):
    """Weight-stationary routed-expert FFN (assumes zero b2).

    Both experts' w1/w2 are preloaded into SBUF (bf16), so the PE never
    waits on weight DMA. Tokens stream in n-groups of 1024 with double
    buffering; each loaded stationary block feeds 2 moving tiles. Per
    expert: L1 over all groups (h kept in SBUF), then L2 over all groups
    (pre-scaled by the combine weight). Long uninterrupted matmul chains
    keep the PE p-state ramped.
    """
    nc = bacc.Bacc("TRN2", target_bir_lowering=False, debug=debug)
    f32 = dt.float32
    bf16 = dt.bfloat16
    W = cfg.EPC * cfg.CAP
    xgT = nc.dram_tensor("xgT", [cfg.D, W], bf16, kind="ExternalInput").ap()
    ew1 = nc.dram_tensor("ew1", [cfg.EPC, cfg.D, cfg.DR], bf16, kind="ExternalInput").ap()
    eb1 = nc.dram_tensor("eb1", [cfg.EPC, cfg.DR], f32, kind="ExternalInput").ap()
    ew2 = nc.dram_tensor("ew2", [cfg.EPC, cfg.DR, cfg.D], bf16, kind="ExternalInput").ap()
    cw = nc.dram_tensor("cw", [cfg.EPC, cfg.CAPP], f32, kind="ExternalInput").ap()
    ygT = nc.dram_tensor("ygT", [cfg.D, W], bf16, kind="ExternalOutput").ap()

    x_po = cfg.D // P
    dr_po = cfg.DR // P
    NG = 1024
    groups = []
    g0 = 0
    while g0 < cfg.CAP:
        gn = min(NG, cfg.CAP - g0)
        groups.append((g0, gn))
        g0 += gn

    with tile.TileContext(nc) as tc, _maybe_loop(tc, loop_n), ExitStack() as ctx:
        const = ctx.enter_context(tc.tile_pool(name="const", bufs=1))
        b1_sb = const.tile([P, cfg.EPC, dr_po], f32)
        nc.sync.dma_start(b1_sb[:], eb1.rearrange("e (po pi) -> pi e po", pi=P))
        cwrep = const.tile([P, cfg.EPC, cfg.CAPP], f32)
        nc.sync.dma_start(
            cwrep[:],
            cw.rearrange("e c -> (e c)")[None].to_broadcast((P, cfg.EPC * cfg.CAPP)),
        )
        w1_sb = const.tile([P, cfg.EPC, x_po, cfg.DR], bf16)
        w2_sb = const.tile([P, cfg.EPC, dr_po, cfg.D], bf16)
        for e in range(cfg.EPC):
            nc.sync.dma_start(
                w1_sb[:, e], ew1[e].rearrange("(po pi) m -> pi po m", pi=P))
            nc.sync.dma_start(
                w2_sb[:, e], ew2[e].rearrange("(po pi) m -> pi po m", pi=P))
        hg = [const.tile([P, dr_po, cfg.CAP], bf16, name=f"hg{e}", tag=f"hg{e}")
              for e in range(cfg.EPC)]

        xpool = ctx.enter_context(tc.tile_pool(name="xg", bufs=2))
        l1ps = ctx.enter_context(tc.tile_pool(name="l1ps", bufs=2, space="PSUM"))
        l2ps = ctx.enter_context(tc.tile_pool(name="l2ps", bufs=2, space="PSUM"))
        stage = ctx.enter_context(tc.tile_pool(name="stage", bufs=6))
        xgT_t = _rearr2(xgT)
        ygT_t = _rearr2(ygT)

        for e in range(cfg.EPC):
            for g0, gn in groups:
                xg = xpool.tile([P, x_po, gn], bf16, tag=f"xg{gn}")
                nc.sync.dma_start(xg[:], xgT_t[:, :, ds(e * cfg.CAP + g0, gn)])
                nts = [(nt * 512, min(512, gn - nt * 512))
                       for nt in range(-(-gn // 512))]
                for m in range(dr_po):
                    ps = [l1ps.tile([P, 512], f32, tag=f"l1p{i}", name=f"l1p{i}")
                          for i in range(len(nts))]
                    for k in range(x_po):
                        for i, (n0, nw) in enumerate(nts):
                            nc.tensor.matmul(
                                ps[i][:, :nw], w1_sb[:, e, k, ts(m, P)],
                                xg[:, k, ds(n0, nw)],
                                start=(k == 0), stop=(k == x_po - 1),
                            )
                    for i, (n0, nw) in enumerate(nts):
                        nc.scalar.activation(
                            hg[e][:, m, ds(g0 + n0, nw)], ps[i][:, :nw],
                            ActFn.Silu, bias=b1_sb[:, e, m:m + 1])
            for g0, gn in groups:
                nts = [(nt * 512, min(512, gn - nt * 512))
                       for nt in range(-(-gn // 512))]
                for mo in range(x_po):
                    ps2 = [l2ps.tile([P, 512], f32, tag=f"l2p{i}", name=f"l2p{i}")
                           for i in range(len(nts))]
                    for k in range(dr_po):
                        for i, (n0, nw) in enumerate(nts):
                            nc.tensor.matmul(
                                ps2[i][:, :nw], w2_sb[:, e, k, ts(mo, P)],
                                hg[e][:, k, ds(g0 + n0, nw)],
                                start=(k == 0), stop=(k == dr_po - 1),
                            )
                    for i, (n0, nw) in enumerate(nts):
                        st = stage.tile([P, 512], bf16, tag="st")
                        nc.vector.tensor_mul(
                            st[:, :nw], ps2[i][:, :nw],
                            cwrep[:, e, ds(g0 + n0, nw)])
                        nc.sync.dma_start(
                            ygT_t[:, mo, ds(e * cfg.CAP + g0 + n0, nw)],
                            st[:, :nw])
    nc.compile()
    return nc


# --------------------------------------------------------------------------
# Fused pipeline: p1f (router + shared L1), p2f (fp8 routed), p3f (L2+combine)
# --------------------------------------------------------------------------

def build_p1f(cfg: Cfg, debug: bool = False, loop_n: int = 0):
    """Router (true-f32 for exact top-2) + shared-expert layer 1.

    The router's x read and PE work hide under the shared-L1 matmul stream;
    x is cast to bf16 on the DVE from the same f32 tiles the router uses.
    h = silu(x @ sw1 + b1) goes to DRAM in bf16 for p3f.
    """
    nc = bacc.Bacc("TRN2", target_bir_lowering=False, debug=debug)
    f32 = dt.float32
    bf16 = dt.bfloat16
    xT = nc.dram_tensor("xT", [cfg.D, cfg.TPC], f32, kind="ExternalInput").ap()
    rw = nc.dram_tensor("rw", [cfg.D, cfg.E], f32, kind="ExternalInput").ap()
    sw1 = nc.dram_tensor("sw1", [cfg.NSH, cfg.D, cfg.DS], bf16, kind="ExternalInput").ap()
    sb1 = nc.dram_tensor("sb1", [cfg.NSH, cfg.DS], f32, kind="ExternalInput").ap()
    ridx = nc.dram_tensor("ridx", [cfg.TPC, 8], dt.uint32, kind="ExternalOutput").ap()
    rwts = nc.dram_tensor("rwts", [cfg.TPC, 8], f32, kind="ExternalOutput").ap()
    hT = nc.dram_tensor("hT", [cfg.NSH, cfg.DS, cfg.TPC], bf16, kind="ExternalOutput").ap()

    x_po = cfg.D // P
    ds_po = cfg.DS // P
    PO_CH = 2
    NPO = x_po // PO_CH
    NG = cfg.TPC // P
    NCH = cfg.TPC // 512  # 512-token chunks

    with tile.TileContext(nc) as tc, _maybe_loop(tc, loop_n), ExitStack() as ctx:
        const = ctx.enter_context(tc.tile_pool(name="const", bufs=1))
        rw_sb = const.tile([P, x_po, cfg.E], f32)
        nc.sync.dma_start(rw_sb[:], rw.rearrange("(po pi) e -> pi po e", pi=P))
        b1_sb = const.tile([P, cfg.NSH, ds_po], f32)
        nc.sync.dma_start(b1_sb[:], sb1.rearrange("s (po pi) -> pi s po", pi=P))
        lg_acc = const.tile([P, NG, cfg.E], f32)
        xbf = const.tile([P, x_po, cfg.TPC], bf16)
        w1_sb = const.tile([P, cfg.NSH, x_po, cfg.DS], bf16)

        xsrc = ctx.enter_context(tc.tile_pool(name="xsrc", bufs=2))
        rps = ctx.enter_context(tc.tile_pool(name="rpsum", bufs=2, space="PSUM"))
        rsb = ctx.enter_context(tc.tile_pool(name="rsb", bufs=3))
        l1ps = ctx.enter_context(tc.tile_pool(name="l1ps", bufs=2, space="PSUM"))
        hstage = ctx.enter_context(tc.tile_pool(name="hst", bufs=4))
        xT_t = _rearr2(xT)

        # ---- router matmuls + bf16 cast of x (PO_CH po-slices at a time) ----
        for po8 in range(NPO):
            xt = xsrc.tile([P, PO_CH, cfg.TPC], f32, tag="xt")
            nc.sync.dma_start(xt[:], xT_t[:, ts(po8, PO_CH), :])
            nc.vector.tensor_copy(xbf[:, ts(po8, PO_CH), :], xt[:])
            for tt in range(NG):
                ps = rps.tile([P, cfg.E], f32, tag="rp")
                for pp in range(PO_CH):
                    nc.tensor.matmul(
                        ps[:], xt[:, pp, ts(tt, P)],
                        rw_sb[:, po8 * PO_CH + pp, :],
                        start=(pp == 0), stop=(pp == PO_CH - 1),
                    )
                if po8 == 0:
                    nc.vector.tensor_copy(lg_acc[:, tt, :], ps[:])
                else:
                    nc.vector.tensor_add(lg_acc[:, tt, :], lg_acc[:, tt, :], ps[:])

        # w1 queued after x so the router never waits on it
        for s in range(cfg.NSH):
            nc.sync.dma_start(
                w1_sb[:, s], sw1[s].rearrange("(po pi) m -> pi po m", pi=P))

        # ---- top-2 softmax ----
        for tt in range(NG):
            t0 = tt * P
            lg = lg_acc[:, tt, :]
            mx = rsb.tile([P, 8], f32, tag="mx")
            nc.vector.max(mx[:], lg)
            ix = rsb.tile([P, 8], dt.uint32, tag="ix")
            nc.vector.max_index(ix[:], mx[:], lg)
            nm = rsb.tile([P, 1], f32, tag="nm")
            nc.vector.tensor_scalar_mul(nm[:], mx[:, 0:1], -1.0)
            ex = rsb.tile([P, cfg.E], f32, tag="ex")
            zz = rsb.tile([P, 1], f32, tag="zz")
            nc.scalar.activation(ex[:], lg, ActFn.Exp, bias=nm[:], accum_out=zz[:])
            rz = rsb.tile([P, 1], f32, tag="rz")
            nc.vector.reciprocal(rz[:], zz[:])
            wv = rsb.tile([P, 8], f32, tag="wv")
            nc.scalar.activation(wv[:], mx[:], ActFn.Exp, bias=nm[:])
            nc.vector.tensor_scalar_mul(wv[:], wv[:], rz[:])
            nc.sync.dma_start(ridx[ds(t0, P), :], ix[:])
            nc.sync.dma_start(rwts[ds(t0, P), :], wv[:])

        # ---- shared layer 1 (weight-stationary, 2-chunk LDW amortization) ----
        hT_t = [hT[s].rearrange("(po pi) t -> pi po t", pi=P)
                for s in range(cfg.NSH)]
        for e in range(cfg.NSH):
            for m in range(ds_po):
                for cp in range(NCH // 2):
                    ps2 = [l1ps.tile([P, 512], f32, tag=f"l1p{i}",
                                     name=f"l1p{i}") for i in range(2)]
                    for k in range(x_po):
                        for i in range(2):
                            nc.tensor.matmul(
                                ps2[i][:], w1_sb[:, e, k, ts(m, P)],
                                xbf[:, k, ds((2 * cp + i) * 512, 512)],
                                start=(k == 0), stop=(k == x_po - 1),
                            )
                    for i in range(2):
                        hst = hstage.tile([P, 512], bf16, tag="hst")
                        nc.scalar.activation(
                            hst[:], ps2[i][:], ActFn.Silu,
                            bias=b1_sb[:, e, m:m + 1])
                        nc.sync.dma_start(
                            hT_t[e][:, m, ds((2 * cp + i) * 512, 512)], hst[:])
    nc.compile()
    return nc


def build_p1g(cfg: Cfg, debug: bool = False, loop_n: int = 0):
    """Router (3-term bf16 split, exact top-2 to ~1e-5) + shared layer 1.

    logits = x_hi·w_hi + x_lo·w_hi + x_hi·w_lo with x_hi/x_lo the bf16
    hi/lo split of f32 x (computed on host).  Router weights are the
    stationary operand so each 512-token group costs 48 N=512 matmuls;
    raw logits go to DRAM and the host does top-2 + softmax exactly.
    h = silu(x @ sw1 + b1) -> DRAM bf16 for p3f.
    """
    nc = bacc.Bacc("TRN2", target_bir_lowering=False, debug=debug)
    f32 = dt.float32
    bf16 = dt.bfloat16
    xbfT = nc.dram_tensor("xbfT", [cfg.D, cfg.TPC], bf16, kind="ExternalInput").ap()
    xloT = nc.dram_tensor("xloT", [cfg.D, cfg.TPC], bf16, kind="ExternalInput").ap()
    rwh = nc.dram_tensor("rwh", [cfg.D, cfg.E], bf16, kind="ExternalInput").ap()
    rwl = nc.dram_tensor("rwl", [cfg.D, cfg.E], bf16, kind="ExternalInput").ap()
    sw1 = nc.dram_tensor("sw1", [cfg.NSH, cfg.D, cfg.DS], bf16, kind="ExternalInput").ap()
    sb1 = nc.dram_tensor("sb1", [cfg.NSH, cfg.DS], f32, kind="ExternalInput").ap()
    lgT = nc.dram_tensor("lgT", [cfg.E, cfg.TPC], f32, kind="ExternalOutput").ap()
    hT = nc.dram_tensor("hT", [cfg.NSH, cfg.DS, cfg.TPC], bf16, kind="ExternalOutput").ap()

    x_po = cfg.D // P
    ds_po = cfg.DS // P
    NG4 = cfg.TPC // 512

    with tile.TileContext(nc) as tc, _maybe_loop(tc, loop_n), ExitStack() as ctx:
        const = ctx.enter_context(tc.tile_pool(name="const", bufs=1))
        rwh_sb = const.tile([P, x_po, cfg.E], bf16)
        rwl_sb = const.tile([P, x_po, cfg.E], bf16)
        b1_sb = const.tile([P, cfg.NSH, ds_po], f32)
        nc.sync.dma_start(b1_sb[:], sb1.rearrange("s (po pi) -> pi s po", pi=P))
        xbf = const.tile([P, x_po, cfg.TPC], bf16)
        w1_sb = const.tile([P, cfg.NSH, x_po, cfg.DS], bf16)

        nc.sync.dma_start(rwh_sb[:], rwh.rearrange("(po pi) e -> pi po e", pi=P))
        nc.sync.dma_start(rwl_sb[:], rwl.rearrange("(po pi) e -> pi po e", pi=P))
        xlop = ctx.enter_context(tc.tile_pool(name="xlo", bufs=1))
        lsb = ctx.enter_context(tc.tile_pool(name="lsb", bufs=3))
        xloT_t = _rearr2(xloT)
        # DMA order: 3 xlo groups (small, unblocks router B-chains), the full
        # xbf, the last xlo group (reusing buffer 0 — its B-chain runs first
        # in group 0 so the buffer frees early), then w1 (L1-only).
        xlo_tiles = [
            xlop.tile([P, x_po, 512], bf16, name=f"xlo{g}", tag=f"xlo{g}")
            for g in range(NG4 - 1)
        ]
        xbfT_t = _rearr2(xbfT)
        # interleave xlo/xbf group slices so router group g is ready at
        # ~6*(g+1) us instead of after the whole 8.4MB xbf load
        for g in range(NG4 - 1):
            nc.sync.dma_start(xlo_tiles[g][:], xloT_t[:, :, ts(g, 512)])
            nc.sync.dma_start(xbf[:, :, ts(g, 512)], xbfT_t[:, :, ts(g, 512)])
        nc.sync.dma_start(xbf[:, :, ts(NG4 - 1, 512)],
                          xbfT_t[:, :, ts(NG4 - 1, 512)])
        xlo_last = xlop.tile([P, x_po, 512], bf16, name="xlo0", tag="xlo0")
        nc.sync.dma_start(xlo_last[:], xloT_t[:, :, ts(NG4 - 1, 512)])
        xlo_tiles.append(xlo_last)
        for s in range(cfg.NSH):
            nc.sync.dma_start(
                w1_sb[:, s], sw1[s].rearrange("(po pi) m -> pi po m", pi=P))

        with ExitStack() as c2:
            rps = c2.enter_context(tc.tile_pool(name="rps", bufs=2, space="PSUM"))
            for g in range(NG4):
                gsl = ts(g, 512)
                # one 48-matmul accumulation group: B + H + L terms
                psR = rps.tile([cfg.E, 512], f32, tag="psR", name="psR")
                for po in range(x_po):
                    nc.tensor.matmul(psR[:], rwh_sb[:, po, :],
                                     xlo_tiles[g][:, po, :],
                                     start=(po == 0), stop=False)
                for po in range(x_po):
                    nc.tensor.matmul(psR[:], rwh_sb[:, po, :], xbf[:, po, gsl],
                                     start=False, stop=False)
                for po in range(x_po):
                    nc.tensor.matmul(psR[:], rwl_sb[:, po, :], xbf[:, po, gsl],
                                     start=False, stop=(po == x_po - 1))
                lg = lsb.tile([cfg.E, 512], f32, tag="lg")
                nc.vector.tensor_copy(lg[:], psR[:])
                nc.sync.dma_start(lgT[:, gsl], lg[:])

        # ---- shared layer 1 (stationary shared across 4 token chunks) ----
        l1ps = ctx.enter_context(tc.tile_pool(name="l1ps", bufs=2, space="PSUM"))
        hstage = ctx.enter_context(tc.tile_pool(name="hst", bufs=4))
        hT_t = [hT[s].rearrange("(po pi) t -> pi po t", pi=P)
                for s in range(cfg.NSH)]
        for e in range(cfg.NSH):
            for m in range(ds_po):
                ps4 = [l1ps.tile([P, 512], f32, tag=f"l1p{i}",
                                 name=f"l1p{i}") for i in range(NG4)]
                for k in range(x_po):
                    for i in range(NG4):
                        nc.tensor.matmul(
                            ps4[i][:], w1_sb[:, e, k, ts(m, P)],
                            xbf[:, k, ts(i, 512)],
                            start=(k == 0), stop=(k == x_po - 1),
                        )
                for i in range(NG4):
                    hst = hstage.tile([P, 512], bf16, tag="hst")
                    nc.scalar.activation(
                        hst[:], ps4[i][:], ActFn.Silu,
                        bias=b1_sb[:, e, m:m + 1])
                    nc.sync.dma_start(hT_t[e][:, m, ts(i, 512)], hst[:])
    nc.compile()
    return nc


def build_p2f(cfg: Cfg, debug: bool = False, loop_n: int = 0):
    """Routed-expert FFN in fp8 (DoubleRow, ~1.44x PE) with per-slot caps.

    Weights arrive pre-scaled by 16 (fp8e4); layer-1 undoes the scale in the
    silu (scale=1/16), layer-2's 16 is folded into cw by the host. Assumes
    zero b2 (the host falls back to the bf16 p2/p2w path otherwise).
    """
    nc = bacc.Bacc("TRN2", target_bir_lowering=False, debug=debug)
    f32 = dt.float32
    bf16 = dt.bfloat16
    fp8 = dt.float8e4
    W = cfg.W2
    caps = (cfg.CAP_A, cfg.CAP_B)
    offs = (0, cfg.CAP_A)
    xgT = nc.dram_tensor("xgT", [cfg.D, W], fp8, kind="ExternalInput").ap()
    ew1 = nc.dram_tensor("ew1", [cfg.EPC, cfg.D, cfg.DR], fp8, kind="ExternalInput").ap()
    eb1 = nc.dram_tensor("eb1", [cfg.EPC, cfg.DR], f32, kind="ExternalInput").ap()
    ew2 = nc.dram_tensor("ew2", [cfg.EPC, cfg.DR, cfg.D], fp8, kind="ExternalInput").ap()
    cw = nc.dram_tensor("cw", [W], f32, kind="ExternalInput").ap()
    ygT = nc.dram_tensor("ygT", [cfg.D, W], bf16, kind="ExternalOutput").ap()

    x_po = cfg.D // P
    dr_po = cfg.DR // P
    DR_MODE = mybir.MatmulPerfMode.DoubleRow

    def chunks(cap):
        out, c0 = [], 0
        while c0 < cap:
            cn = min(512, cap - c0)
            out.append((c0, cn))
            c0 += cn
        return out

    with tile.TileContext(nc) as tc, _maybe_loop(tc, loop_n), ExitStack() as ctx:
        const = ctx.enter_context(tc.tile_pool(name="const", bufs=1))
        b1_sb = const.tile([P, cfg.EPC, dr_po], f32)
        nc.sync.dma_start(b1_sb[:], eb1.rearrange("e (po pi) -> pi e po", pi=P))
        cwrep = const.tile([P, W], f32)
        nc.sync.dma_start(cwrep[:], cw[None].to_broadcast((P, W)))
        w1_sb = const.tile([P, cfg.EPC, x_po, cfg.DR], fp8)
        w2_sb = const.tile([P, cfg.EPC, dr_po, cfg.D], fp8)
        hg = [const.tile([P, dr_po, caps[e]], fp8, name=f"hg{e}", tag=f"hg{e}")
              for e in range(cfg.EPC)]

        xpool = ctx.enter_context(tc.tile_pool(name="xg", bufs=3))
        l1ps = ctx.enter_context(tc.tile_pool(name="l1ps", bufs=2, space="PSUM"))
        l2ps = ctx.enter_context(tc.tile_pool(name="l2ps", bufs=2, space="PSUM"))
        stage = ctx.enter_context(tc.tile_pool(name="stage", bufs=6))
        xgT_t = _rearr2(xgT)
        ygT_t = _rearr2(ygT)

        for e in range(cfg.EPC):
            nc.sync.dma_start(
                w1_sb[:, e], ew1[e].rearrange("(po pi) m -> pi po m", pi=P))
        for e in range(cfg.EPC):
            nc.sync.dma_start(
                w2_sb[:, e], ew2[e].rearrange("(po pi) m -> pi po m", pi=P))

        # ---- layer 1 (DoubleRow: contract 256 rows per matmul) ----
        for e in range(cfg.EPC):
            for g0, gn in chunks(caps[e]):
                xg = xpool.tile([P, x_po, gn], fp8, tag=f"xg{gn}")
                nc.sync.dma_start(xg[:], xgT_t[:, :, ds(offs[e] + g0, gn)])
                for m in range(dr_po):
                    ps = l1ps.tile([P, 512], f32, tag="l1p")
                    for k2 in range(x_po // 2):
                        nc.tensor.matmul(
                            ps[:, :gn], w1_sb[:, e, ds(2 * k2, 2), ts(m, P)],
                            xg[:, ds(2 * k2, 2), :],
                            start=(k2 == 0), stop=(k2 == x_po // 2 - 1),
                            perf_mode=DR_MODE,
                        )
                    nc.scalar.activation(
                        hg[e][:, m, ds(g0, gn)], ps[:, :gn], ActFn.Silu,
                        bias=b1_sb[:, e, m:m + 1], scale=1.0 / 16.0)

        # ---- layer 2 (DoubleRow over DR; output pre-scaled by cw/16) ----
        for e in range(cfg.EPC):
            for g0, gn in chunks(caps[e]):
                for mo in range(x_po):
                    ps = l2ps.tile([P, 512], f32, tag="l2p")
                    for k2 in range(dr_po // 2):
                        nc.tensor.matmul(
                            ps[:, :gn], w2_sb[:, e, ds(2 * k2, 2), ts(mo, P)],
                            hg[e][:, ds(2 * k2, 2), ds(g0, gn)],
                            start=(k2 == 0), stop=(k2 == dr_po // 2 - 1),
                            perf_mode=DR_MODE,
                        )
                    st = stage.tile([P, 512], bf16, tag="st")
                    nc.vector.tensor_mul(
                        st[:, :gn], ps[:, :gn],
                        cwrep[:, ds(offs[e] + g0, gn)])
                    nc.sync.dma_start(
                        ygT_t[:, mo, ds(offs[e] + g0, gn)], st[:, :gn])
    nc.compile()
    return nc


def build_p3f(cfg: Cfg, debug: bool = False, loop_n: int = 0, has_b2: bool = False):
    """Shared layer 2 + combine, f32 accumulation and f32 output.

    out = h @ sw2 (+ b2sum) + ya + yb with the adds on the DVE reading the
    f32 PSUM directly; the only low-precision steps left are the bf16/fp8
    matmul operands themselves.
    """
    nc = bacc.Bacc("TRN2", target_bir_lowering=False, debug=debug)
    f32 = dt.float32
    bf16 = dt.bfloat16
    hT = nc.dram_tensor("hT", [cfg.NSH, cfg.DS, cfg.TPC], bf16, kind="ExternalInput").ap()
    sw2 = nc.dram_tensor("sw2", [cfg.NSH, cfg.DS, cfg.D], bf16, kind="ExternalInput").ap()
    sb2 = nc.dram_tensor("sb2", [cfg.NSH, cfg.D], f32, kind="ExternalInput").ap()
    yaT = nc.dram_tensor("yaT", [cfg.D, cfg.TPC], bf16, kind="ExternalInput").ap()
    ybT = nc.dram_tensor("ybT", [cfg.D, cfg.TPC], bf16, kind="ExternalInput").ap()
    outT = nc.dram_tensor("outT", [cfg.D, cfg.TPC], f32, kind="ExternalOutput").ap()

    x_po = cfg.D // P
    ds_po = cfg.DS // P
    NCH = cfg.TPC // 512

    with tile.TileContext(nc) as tc, _maybe_loop(tc, loop_n), ExitStack() as ctx:
        const = ctx.enter_context(tc.tile_pool(name="const", bufs=1))
        b2_sb = const.tile([P, cfg.NSH, x_po], f32)
        nc.sync.dma_start(b2_sb[:], sb2.rearrange("s (po pi) -> pi s po", pi=P))
        b2sum = const.tile([P, x_po], f32)
        nc.vector.tensor_add(b2sum[:], b2_sb[:, 0], b2_sb[:, 1])
        h_sb = const.tile([P, cfg.NSH, ds_po, cfg.TPC], bf16)
        w2_sb = const.tile([P, cfg.NSH, ds_po, cfg.D], bf16)
        # interleave (h, w2) k-slices so the first contraction chain can
        # start as soon as the first slices land
        for e in range(cfg.NSH):
            hT_e = hT[e].rearrange("(po pi) t -> pi po t", pi=P)
            w2_e = sw2[e].rearrange("(po pi) m -> pi po m", pi=P)
            for k in range(ds_po):
                nc.sync.dma_start(h_sb[:, e, k], hT_e[:, k])
                nc.sync.dma_start(w2_sb[:, e, k], w2_e[:, k])

        l2ps = ctx.enter_context(tc.tile_pool(name="l2ps", bufs=1, space="PSUM"))
        ypool = ctx.enter_context(tc.tile_pool(name="yp", bufs=2))
        stage = ctx.enter_context(tc.tile_pool(name="stage", bufs=4))
        yaT_t, ybT_t, outT_t = _rearr2(yaT), _rearr2(ybT), _rearr2(outT)

        # Per-expert PSUM chains (e0 can start once its h/w2 slices land) and
        # the stationary shared across all 4 token chunks (LDW amortization).
        for mo in range(x_po):
            yts = []
            for i in range(NCH):
                ya_t = ypool.tile([P, 512], f32, tag=f"ya{i}")
                nc.gpsimd.dma_start(ya_t[:], yaT_t[:, mo, ts(i, 512)])
                yb_t = ypool.tile([P, 512], f32, tag=f"yb{i}")
                nc.gpsimd.dma_start(yb_t[:], ybT_t[:, mo, ts(i, 512)])
                yts.append((ya_t, yb_t))
            ps = [[l2ps.tile([P, 512], f32, tag=f"l2p{e}_{i}",
                             name=f"l2p{e}_{i}") for i in range(NCH)]
                  for e in range(cfg.NSH)]
            for e in range(cfg.NSH):
                for k in range(ds_po):
                    for i in range(NCH):
                        nc.tensor.matmul(
                            ps[e][i][:], w2_sb[:, e, k, ts(mo, P)],
                            h_sb[:, e, k, ts(i, 512)],
                            start=(k == 0), stop=(k == ds_po - 1),
                        )
            for i in range(NCH):
                st = stage.tile([P, 512], f32, tag="st")
                nc.vector.tensor_add(st[:], ps[0][i][:], yts[i][0][:])
                nc.vector.tensor_add(st[:], st[:], yts[i][1][:])
                nc.vector.tensor_add(st[:], st[:], ps[1][i][:])
                if has_b2:
                    nc.vector.tensor_scalar_add(
                        st[:], st[:], b2sum[:, mo:mo + 1])
                nc.sync.dma_start(outT_t[:, mo, ts(i, 512)], st[:])
    nc.compile()
    return nc


# --------------------------------------------------------------------------
# Phase 3: combine out = shared + y0 + y1
# --------------------------------------------------------------------------

def build_p3(cfg: Cfg, debug: bool = False, loop_n: int = 0):
    nc = bacc.Bacc("TRN2", target_bir_lowering=False, debug=debug)
    f32 = dt.float32
    aT = nc.dram_tensor("aT", [cfg.D, cfg.TPC], f32, kind="ExternalInput").ap()
    bT = nc.dram_tensor("bT", [cfg.D, cfg.TPC], dt.bfloat16, kind="ExternalInput").ap()
    cT = nc.dram_tensor("cT", [cfg.D, cfg.TPC], dt.bfloat16, kind="ExternalInput").ap()
    oT = nc.dram_tensor("oT", [cfg.D, cfg.TPC], f32, kind="ExternalOutput").ap()

    x_po = cfg.D // P
    CH = 128
    with tile.TileContext(nc) as tc, _maybe_loop(tc, loop_n), ExitStack() as ctx:
        pool = ctx.enter_context(tc.tile_pool(name="sb", bufs=3))
        aT_t, bT_t, cT_t, oT_t = _rearr2(aT), _rearr2(bT), _rearr2(cT), _rearr2(oT)
        for c in range(cfg.TPC // CH):
            a = pool.tile([P, x_po, CH], f32, tag="a")
            nc.sync.dma_start(a[:], aT_t[:, :, ts(c, CH)])
            b = pool.tile([P, x_po, CH], f32, tag="b")
            nc.gpsimd.dma_start(b[:], bT_t[:, :, ts(c, CH)])  # bf16 -> f32 cast
            cc = pool.tile([P, x_po, CH], f32, tag="c")
            nc.gpsimd.dma_start(cc[:], cT_t[:, :, ts(c, CH)])
            nc.vector.tensor_add(a[:], a[:], b[:])
            nc.vector.tensor_add(a[:], a[:], cc[:])
            nc.sync.dma_start(oT_t[:, :, ts(c, CH)], a[:])
    nc.compile()
    return nc


# --------------------------------------------------------------------------
# Host orchestration
# --------------------------------------------------------------------------

def _get(phase: str, cfg: Cfg, **bkw):
    key = (phase, cfg, tuple(sorted(bkw.items())))
    if key not in _cache:
        _cache[key] = {
            "p1": build_p1, "p2": build_p2, "p3": build_p3,
            "p1r": build_p1r, "p3s": build_p3s, "pr": build_pr,
            "p2w": build_p2w,
            "p1f": build_p1f, "p2f": build_p2f, "p3f": build_p3f,
            "p1g": build_p1g,
        }[phase](cfg, **bkw)
    return _cache[key]


def _run(phase: str, cfg: Cfg, in_maps, **bkw):
    nc = _get(phase, cfg, **bkw)
    r = run_bass_kernel_spmd(nc, in_maps, core_ids=list(range(cfg.n_cores)), trace=TRACE)
    LAST_EXEC_NS[phase] = r.exec_time_ns
    return r.results


def kernel(**inputs) -> np.ndarray:
    cfg = CFG
    if np.any(np.asarray(inputs["re_b2"])):
        return _kernel_fallback(**inputs)
    E4NP = dt.np(dt.float8e4)
    x = np.ascontiguousarray(np.asarray(inputs["x"], dtype=np.float32))
    Bn, S, D = x.shape
    assert (Bn, S, D) == (cfg.n_cores, cfg.TPC, cfg.D)
    step_t = int(np.asarray(inputs["step_t"]))
    rw = np.ascontiguousarray(np.asarray(inputs["router_w"], np.float32)[step_t])
    re_b1 = np.ascontiguousarray(np.asarray(inputs["re_b1"], np.float32))
    w1f8 = np.ascontiguousarray(
        (np.asarray(inputs["re_w1"], np.float32) * 16.0).astype(E4NP))
    w2f8 = np.ascontiguousarray(
        (np.asarray(inputs["re_w2"], np.float32) * 16.0).astype(E4NP))
    sh_w1 = np.ascontiguousarray(np.asarray(inputs["sh_w1"], np.float32).astype(BF16))
    sh_b1 = np.ascontiguousarray(np.asarray(inputs["sh_b1"], np.float32))
    sh_w2 = np.ascontiguousarray(np.asarray(inputs["sh_w2"], np.float32).astype(BF16))
    sh_b2 = np.ascontiguousarray(np.asarray(inputs["sh_b2"], np.float32))

    xT = np.ascontiguousarray(x.transpose(0, 2, 1))  # [B, D, S] feature-major
    xbf = xT.astype(BF16)                            # hi half (bf16)
    xlo = (xT - xbf.astype(np.float32)).astype(BF16)  # lo half (bf16)
    rwh = rw.astype(BF16)
    rwl = (rw - rwh.astype(np.float32)).astype(BF16)

    # ---- phase 1: router logits + shared layer 1 ----
    in1 = [{"xbfT": np.ascontiguousarray(xbf[b]),
            "xloT": np.ascontiguousarray(xlo[b]),
            "rwh": rwh, "rwl": rwl, "sw1": sh_w1, "sb1": sh_b1}
           for b in range(cfg.n_cores)]
    r1 = _run("p1g", cfg, in1)

    # host top-2 + softmax on the exact device logits
    logits = np.stack([r["lgT"].T for r in r1])       # [B, S, E] f32
    idx = np.argsort(-logits, axis=-1, kind="stable")[..., :2].astype(np.int64)
    mx = logits.max(-1, keepdims=True)
    el = np.exp(logits - mx)
    probs = el / el.sum(-1, keepdims=True)
    wts = np.take_along_axis(probs, idx, axis=-1).astype(np.float32)  # [B, S, 2]

    T = Bn * S
    pair_e = idx.reshape(-1)                   # expert of pair p (p = g*2 + k)
    order = np.argsort(pair_e, kind="stable")  # pairs sorted by expert
    counts = np.bincount(pair_e, minlength=cfg.E)

    # slot assignment: 8 most-loaded experts -> slot A, rest -> slot B
    eorder = np.argsort(-counts, kind="stable")
    slotA, slotB = eorder[:cfg.n_cores], eorder[cfg.n_cores:]
    pad512 = lambda n: max(512, int(-(-int(n) // 512) * 512))
    if counts[slotA].max() > cfg.CAP_A or counts[slotB].max() > cfg.CAP_B:
        cfg = Cfg(CAP_A=pad512(counts[slotA].max()),
                  CAP_B=pad512(counts[slotB].max()))
    W = cfg.W2
    core_of = np.empty(cfg.E, np.int64)
    slot_off = np.empty(cfg.E, np.int64)
    core_of[slotA] = np.arange(cfg.n_cores); slot_off[slotA] = 0
    core_of[slotB] = np.arange(cfg.n_cores); slot_off[slotB] = cfg.CAP_A

    xball = np.concatenate(
        [xT[b].astype(BF16) for b in range(Bn)], axis=1).astype(E4NP)  # [D, T]

    seg = np.zeros(cfg.E + 1, np.int64)
    seg[1:] = np.cumsum(counts)
    caps = np.where(slot_off == 0, cfg.CAP_A, cfg.CAP_B)
    cols = [np.zeros(int(caps[e]), np.int64) for e in range(cfg.E)]
    cwv = [np.zeros(int(caps[e]), np.float32) for e in range(cfg.E)]
    pos_of_pair = np.empty(2 * T, np.int64)
    wflat = wts.reshape(-1)
    for e in range(cfg.E):
        sl = order[seg[e]:seg[e + 1]]
        n = len(sl)
        cols[e][:n] = sl // 2
        cwv[e][:n] = wflat[sl] / 16.0          # fold w2's 16x fp8 scale
        pos_of_pair[sl] = np.arange(n)

    # ---- phase 2: routed experts (fp8) ----
    in2 = []
    for c in range(cfg.n_cores):
        eA, eB = int(slotA[c]), int(slotB[c])
        xg = xball[:, np.concatenate([cols[eA], cols[eB]])]
        in2.append({
            "xgT": np.ascontiguousarray(xg),
            "ew1": w1f8[[eA, eB]],
            "eb1": re_b1[[eA, eB]],
            "ew2": w2f8[[eA, eB]],
            "cw": np.concatenate([cwv[eA], cwv[eB]]),
        })
    r2 = _run("p2f", cfg, in2)

    yall = np.concatenate([r["ygT"] for r in r2], axis=1)  # [D, B*W] bf16
    ycol_of_pair = core_of[pair_e] * W + slot_off[pair_e] + pos_of_pair
    ya = yall[:, ycol_of_pair[0::2]]                       # [D, T] slot k=0
    yb = yall[:, ycol_of_pair[1::2]]                       # [D, T] slot k=1

    # ---- phase 3: shared layer 2 + combine ----
    in3 = [
        {
            "hT": r1[b]["hT"],
            "sw2": sh_w2, "sb2": sh_b2,
            "yaT": np.ascontiguousarray(ya[:, b * S:(b + 1) * S]),
            "ybT": np.ascontiguousarray(yb[:, b * S:(b + 1) * S]),
        }
        for b in range(cfg.n_cores)
    ]
    r3 = _run("p3f", cfg, in3, has_b2=bool(np.any(sh_b2)))

    out = np.stack([r["outT"] for r in r3])                # [B, D, S] f32
    return np.ascontiguousarray(out.transpose(0, 2, 1))   # [B, S, D] f32


def _kernel_fallback(**inputs) -> np.ndarray:
    cfg = CFG
    x = np.ascontiguousarray(np.asarray(inputs["x"], dtype=np.float32))
    Bn, S, D = x.shape
    assert (Bn, S, D) == (cfg.n_cores, cfg.TPC, cfg.D)
    step_t = int(np.asarray(inputs["step_t"]))
    rw = np.ascontiguousarray(np.asarray(inputs["router_w"], np.float32)[step_t])
    re_w1 = np.ascontiguousarray(np.asarray(inputs["re_w1"], np.float32).astype(BF16))
    re_b1 = np.ascontiguousarray(np.asarray(inputs["re_b1"], np.float32))
    re_w2 = np.ascontiguousarray(np.asarray(inputs["re_w2"], np.float32).astype(BF16))
    re_b2 = np.ascontiguousarray(np.asarray(inputs["re_b2"], np.float32))
    sh_w1 = np.ascontiguousarray(np.asarray(inputs["sh_w1"], np.float32).astype(BF16))
    sh_b1 = np.ascontiguousarray(np.asarray(inputs["sh_b1"], np.float32))
    sh_w2 = np.ascontiguousarray(np.asarray(inputs["sh_w2"], np.float32).astype(BF16))
    sh_b2 = np.ascontiguousarray(np.asarray(inputs["sh_b2"], np.float32))

    xT = np.ascontiguousarray(x.transpose(0, 2, 1))  # [B, D, S] feature-major
    xbfT = xT.astype(BF16)                           # device compute dtype

    # ---- phase 1: router ----
    in1 = [{"xT": xT[b], "rw": rw} for b in range(cfg.n_cores)]
    r1 = _run("pr", cfg, in1)

    idx = np.stack([r["ridx"][:, :2] for r in r1]).astype(np.int64)   # [B, S, 2]
    wts = np.stack([r["rwts"][:, :2] for r in r1])                    # [B, S, 2] f32
    xball = np.concatenate(list(xbfT), axis=1)                        # [D, T] bf16

    T = Bn * S
    pair_e = idx.reshape(-1)                   # expert of pair p (p = g*2 + k)
    order = np.argsort(pair_e, kind="stable")  # pairs sorted by expert
    counts = np.bincount(pair_e, minlength=cfg.E)

    if counts.max() > cfg.CAP:  # safety net: regrow capacity, rebuild p2
        cfg = Cfg(CAP=int(-(-(counts.max() + 64) // P) * P))

    seg = np.zeros(cfg.E + 1, np.int64)
    seg[1:] = np.cumsum(counts)
    cols = np.zeros((cfg.E, cfg.CAP), np.int64)           # token col in xball
    cwarr = np.zeros((cfg.E, cfg.CAPP), np.float32)       # combine weights
    pos_of_pair = np.empty(2 * T, np.int64)
    wflat = wts.reshape(-1)
    for e in range(cfg.E):
        sl = order[seg[e]:seg[e + 1]]
        n = len(sl)
        cols[e, :n] = sl // 2
        cwarr[e, :n] = wflat[sl]
        pos_of_pair[sl] = np.arange(n)

    # ---- phase 2 ----
    in2 = []
    for c in range(cfg.n_cores):
        e0 = c * cfg.EPC
        xg = xball[:, cols[e0:e0 + cfg.EPC].reshape(-1)]  # [D, EPC*CAP] bf16
        in2.append({
            "xgT": np.ascontiguousarray(xg),
            "ew1": re_w1[e0:e0 + cfg.EPC],
            "eb1": re_b1[e0:e0 + cfg.EPC],
            "ew2": re_w2[e0:e0 + cfg.EPC],
            "eb2": re_b2[e0:e0 + cfg.EPC],
            "cw": cwarr[e0:e0 + cfg.EPC],
        })
    has_b2 = bool(np.any(np.asarray(inputs["re_b2"])))
    if has_b2:
        r2 = _run("p2", cfg, in2, has_b2=True)
    else:
        for m in in2:
            m.pop("eb2")
        r2 = _run("p2w", cfg, in2)

    # global y layout: expert e occupies columns [e*CAP, (e+1)*CAP)
    yall = np.concatenate([r["ygT"] for r in r2], axis=1)  # [D, E*CAP] bf16

    ycol_of_pair = pair_e * cfg.CAP + pos_of_pair          # [2T]
    ya = yall[:, ycol_of_pair[0::2]]                       # [D, T] slot k=0
    yb = yall[:, ycol_of_pair[1::2]]                       # [D, T] slot k=1

    # ---- phase 3: shared experts + combine ----
    in3 = [
        {
            "xbfT": xbfT[b],
            "sw1": sh_w1, "sb1": sh_b1, "sw2": sh_w2, "sb2": sh_b2,
            "yaT": np.ascontiguousarray(ya[:, b * S:(b + 1) * S]),
            "ybT": np.ascontiguousarray(yb[:, b * S:(b + 1) * S]),
        }
        for b in range(cfg.n_cores)
    ]
    r3 = _run("p3s", cfg, in3, has_b2=bool(np.any(sh_b2)))

    out = np.stack([r["outT"] for r in r3]).astype(np.float32)  # [B, D, S]
    return np.ascontiguousarray(out.transpose(0, 2, 1))    # [B, S, D] f32

